# revision 59
# baseline (speedup 1.0000x reference)
"""Trainium2 Bass kernel for nn_MultiHeadCrossAttention.

Reference computation (B=2, S=2048, D=1024, H=16, HD=64):
  Qv,Kv,Vv = vis @ W_{q,k,v}_vis + b ; Qi,Ki,Vi = inf @ W_{q,k,v}_inf + b
  out_inf = softmax(Qv Ki^T / 8) Vi @ W_o_inf + b_o_inf
  out_vis = softmax(Qi Kv^T / 8) Vv @ W_o_vis + b_o_vis

Sharding: tensor-parallel over the 16 heads; core c owns heads 2c, 2c+1
(columns 128c:128c+128 of the QKV projections, rows of W_o). Each core
computes a full-shape partial of both outputs; the host sums the 8
partials (the "all-reduce after fc_out") and adds the output biases.

Device dataflow is fully transposed (token dim on the free axis):
  QT/KT/VT[j, t] = W.T @ X^T        (W stationary, X^T moving, 8 K-tiles)
  V = transpose(VT) via PE          (+ ones column -> V_aug [128k, 65])
  S^T[k, q]      = KT.T @ QT        (per head, K=64, row-group packed:
                                     both heads' matmuls run concurrently
                                     in disjoint PE row groups)
  E = exp(0.125 * S^T)              (ScalarE, PSUM -> SBUF bf16)
  PV[hd+1, q]    = V_aug.T @ E      (K=128; row 64 = softmax denominator)
  A^T[j, q]      = PV[:64] * bcast(1/PV[64])
  OUT^T[m, t]    = Wo.T @ A^T       (K=128, 8 m-tiles, bf16 partials out)

Schedule: ONE global software pipeline.  The ScalarE exp stream (256
ACTIVATEs x ~1.05us engine-busy = the largest single-engine load) is the
master sequence; all other PE work is filler popped into PE slack
between attention matmuls.  Filler lives in three queues ordered by
consumption time:
  fq_kv: K/V projections + V transposes (a phase's kv side is consumed
         entirely within its FIRST query tile, so these are urgent),
  fq_q:  Q projections (consumed one tile per query tile — relaxed;
         loads its own copy of the input tiles so the two queues share
         no SBUF ring and can pop independently),
  fq_wo: deferred Wo output-projection tiles (no deadline).
ensure() force-pops a queue up to a dependency watermark before any
attention matmul that consumes it is emitted, which both guarantees
deadlock-freedom (every engine queue's order embeds in one global
topological order) and makes the schedule self-correcting.

PSUM budget (8 banks): scores pair [128,2,512]f32 x2 bufs = 4, PV pair
[65,512]f32 x2 = 2, projection accumulator = 1, Wo-out/transpose = 1.
"""

import sys
from collections import deque

for _p in ("/opt/trn_rl_repo", "/root/.axon_site/_ro/trn_rl_repo"):
    if _p not in sys.path:
        sys.path.append(_p)

import numpy as np
import ml_dtypes

import concourse.bass as bass
import concourse.tile as tile
from concourse import bacc, mybir
from concourse.masks import make_identity

F32 = mybir.dt.float32
BF16 = mybir.dt.bfloat16
EXP = mybir.ActivationFunctionType.Exp

B, S, D, H = 2, 2048, 1024, 16
HD = 64
JC = 128          # head dims per core (2 heads x 64)
N_CORES = 8
NT = 512          # token tile (moving dim) for projections
NQ = 512          # query tile for attention
DKT = D // 128    # 8 contraction tiles for projections
SCALE = 1.0 / np.sqrt(HD)

NTT = S // NT     # 4 token tiles per batch
NQT = S // NQ     # 4 query tiles
NKT = S // 128    # 16 key tiles
NMT = D // 128    # 8 output m-tiles

PV_LAG = 3        # PV trails the scores/exp stream by 3 key tiles
POPS_PER_TILE = 2.6


def build_kernel():
    nc = bacc.Bacc()

    visT = nc.dram_tensor("visT", [B, D, S], BF16, kind="ExternalInput")
    infT = nc.dram_tensor("infT", [B, D, S], BF16, kind="ExternalInput")
    w_in = {}
    b_in = {}
    for st in ("v", "i"):
        for p in ("q", "k", "v"):
            w_in[p + st] = nc.dram_tensor(f"w_{p}{st}", [D, JC], BF16, kind="ExternalInput")
            b_in[p + st] = nc.dram_tensor(f"b_{p}{st}", [JC], F32, kind="ExternalInput")
    w_ov = nc.dram_tensor("w_ov", [JC, D], BF16, kind="ExternalInput")
    w_oi = nc.dram_tensor("w_oi", [JC, D], BF16, kind="ExternalInput")
    # bf16 partials: the host sums them in f32, so the only cost is one
    # rounding of each partial (~3e-4 relative on the summed output).
    o_vis = nc.dram_tensor("o_vis", [B, D, S], BF16, kind="ExternalOutput")
    o_inf = nc.dram_tensor("o_inf", [B, D, S], BF16, kind="ExternalOutput")

    with tile.TileContext(nc) as tc:
        with (
            tc.tile_pool(name="const", bufs=1) as cpool,
            tc.tile_pool(name="wpool", bufs=1) as wpool,
            tc.tile_pool(name="proj", bufs=1) as projpool,
            tc.tile_pool(name="xin", bufs=2) as xpool,
            tc.tile_pool(name="esb", bufs=6) as epool,
            tc.tile_pool(name="small", bufs=2) as spool,
            tc.tile_pool(name="outst", bufs=4) as opool,
            tc.tile_pool(name="ps", bufs=1, space="PSUM") as ps,
        ):
            ident = cpool.tile([128, 128], BF16)
            make_identity(nc, ident[:])

            # Pre-load the exp table set (~2.7us) under the prologue DMAs
            # instead of on the first real scores tile.
            warm = cpool.tile([1, 1], F32, tag="warm", name="warm")
            nc.vector.memset(warm[:], 0.0)
            nc.scalar.activation(warm[:], warm[:], EXP)

            def pe_warm_spin():
                # Dummy matmuls bridging the ~13us wait for the first
                # input DMAs: keeps the HAM activity window tripped so
                # the first real matmuls run at 2.4GHz, not 1.2GHz.
                # Alternate two PSUM banks so the WAW chain still
                # pipelines.
                wa = ps.tile([128, 128], F32, tag="acc", bufs=1,
                             name="warmpa")
                wb = ps.tile([65, 128], F32, tag="pv0", name="warmpb")
                for _ in range(36):
                    nc.tensor.matmul(wa[:], ident[:], ident[:],
                                     start=True, stop=True)
                    nc.tensor.matmul(wb[:], ident[:, 0:65], ident[:],
                                     start=True, stop=True)

            # Weight/bias DMAs emitted lazily at first use.
            _w_tiles, _b_tiles, _wo_tiles = {}, {}, {}

            def w_sb_get(key, eng=None):
                # Prologue weight loads post from the still-idle qACT DGE
                # queue so they run in parallel with qSP's input postings
                # (descriptor postings are the scarce serial resource).
                if key not in _w_tiles:
                    e = eng or nc.sync
                    t = wpool.tile([128, DKT, JC], BF16, tag=f"w_{key}",
                                   name=f"w_{key}")
                    src = w_in[key].rearrange("(kt p) j -> p kt j", p=128)
                    for h in range(0, DKT, 2):
                        e.dma_start(t[:, h:h + 2, :], src[:, h:h + 2, :])
                    _w_tiles[key] = t
                return _w_tiles[key]

            # All six QKV biases in one batch: six contiguous [1,128] row
            # DMAs (a [128,1] gather costs ~3.5us of descriptor posting
            # each), one cast, ONE K=6 matmul against I6 to flip them to
            # [128, 6], one copy out.  bias_sb_get returns column slices.
            _BKEYS = ("ki", "vi", "qv", "kv", "vv", "qi")
            _bias_state = {}

            def load_biases():
                rows = cpool.tile([6, JC], F32, tag="brows", name="brows")
                for idx, key in enumerate(_BKEYS):
                    nc.scalar.dma_start(rows[idx:idx + 1, :],
                                        b_in[key][:].unsqueeze(0))
                rows16 = cpool.tile([6, JC], BF16, tag="brows16",
                                    name="brows16")
                nc.vector.tensor_copy(rows16[:], rows[:])
                tp = ps.tile([JC, 6], F32, tag="pwo", bufs=1, name="btp")
                nc.tensor.matmul(tp[:], rows16[:], ident[0:6, 0:6],
                                 start=True, stop=True)
                ball = cpool.tile([JC, 6], F32, tag="ball", name="ball")
                nc.vector.tensor_copy(ball[:], tp[:])
                _bias_state["ball"] = ball

            def bias_sb_get(key):
                idx = _BKEYS.index(key)
                return _bias_state["ball"][:, idx:idx + 1]

            def wo_sb_get(key):
                if key not in _wo_tiles:
                    wd = {"v": w_ov, "i": w_oi}[key]
                    t = wpool.tile([JC, NMT, 128], BF16, tag=f"wo_{key}",
                                   name=f"wo_{key}")
                    nc.sync.dma_start(
                        t[:], wd.rearrange("j (mt m) -> j mt m", m=128))
                    _wo_tiles[key] = t
                return _wo_tiles[key]

            xT = {"v": visT, "i": infT}
            o_dram = {"v": o_vis, "i": o_inf}

            # ---- per-(b, st) projection output tiles --------------------
            # Each tag ring has bufs=2 and exactly two allocations (b=0,
            # b=1), so the two batches never alias.
            qt_sb, kt_sb, vt_sb, vaug_sb = {}, {}, {}, {}

            def get_proj_tiles(b, st):
                key = (b, st)
                if key not in qt_sb:
                    qt_sb[key] = projpool.tile([JC, S], BF16, tag=f"QT_{st}",
                                               bufs=2, name=f"QT_{st}{b}")
                    kt_sb[key] = projpool.tile([JC, S], BF16, tag=f"KT_{st}",
                                               bufs=2, name=f"KT_{st}{b}")
                    vt_sb[key] = projpool.tile([JC, S], BF16, tag=f"VT_{st}",
                                               bufs=2, name=f"VT_{st}{b}")
                    vaug_sb[key] = projpool.tile([128, NKT, 130], BF16,
                                                 tag=f"Vaug_{st}", bufs=2,
                                                 name=f"Vaug_{st}{b}")

            # ---- filler task queues -------------------------------------
            # Credit-based popping: each attention tile adds
            # POPS_PER_TILE of PE-work credit; popped units (including
            # ensure()-forced ones) consume it, so dependency bursts
            # automatically suppress later discretionary pops.  Units are
            # ATOMIC: a whole PSUM accumulation group lives in one unit,
            # so ring-shared PSUM tags are never interleaved mid-group.
            fq_kv = deque()
            fq_q = deque()
            fq_wo = deque()
            _done = set()
            _credit = [0.0]
            _rr = [0]

            def _pop_one(q):
                w, fn, provides = q.popleft()
                fn()
                if provides is not None:
                    _done.add(provides)
                _credit[0] -= max(w, 0.25)

            def add_credit(n):
                _credit[0] = min(_credit[0] + n, 8.0)

            def pop_filler():
                while _credit[0] > 0 and (fq_kv or fq_q or fq_wo):
                    # Drain the Wo backlog preferentially once it builds,
                    # else it all lands in a serial post-attention tail.
                    if len(fq_wo) > 3:
                        _pop_one(fq_wo)
                        continue
                    for _ in range(3):
                        q = (fq_kv, fq_q, fq_wo)[_rr[0] % 3]
                        _rr[0] += 1
                        if q:
                            _pop_one(q)
                            break

            def ensure(key):
                q = fq_q if (key[0] == "P" and key[3] == "q") else fq_kv
                while key not in _done:
                    assert q, f"dependency {key} not queued"
                    _pop_one(q)

            def flush_filler():
                for q in (fq_kv, fq_q, fq_wo):
                    while q:
                        _pop_one(q)

            # ---- projection filler units --------------------------------
            # side: "kv" tiles feed K/V projections, "q" tiles feed Q.
            # Separate tags so the two queues never share an SBUF ring
            # (the q side re-loads the inputs; DMA bandwidth is cheap).
            xt_live = {}
            acc_live = {}

            def u_dma_x(side, b, st, tt, eng=None):
                # Per-kt 2D transfers: each [128, 512] piece is one clean
                # descriptor posting with its own completion, so the
                # projection matmuls for early kt can start while later
                # pieces are still in flight.
                def fn():
                    e = eng or nc.sync
                    xt = xpool.tile([128, DKT, NT], BF16, tag=f"x{side}_{st}",
                                    bufs=(3 if side == "kv" else 1),
                                    name=f"x{side}_{st}")
                    src = xT[st].rearrange("bb (kt p) t -> bb p kt t", p=128)[
                        b, :, :, tt * NT:(tt + 1) * NT]
                    for h in range(DKT):
                        e.dma_start(xt[:, h, :], src[:, h, :])
                    xt_live[(side, b, st, tt)] = xt
                return (0.0, fn, None)

            def u_proj_kv_half(b, st, tt, p, half):
                # half a K/V projection group (4 accumulating matmuls).
                # The "acc" bank is used ONLY by these kv units (plus
                # tail-time Wo units once the proj queues are empty), and
                # both halves sit adjacent in fq_kv, so no other acc
                # allocation can interleave the accumulation group.
                def fn():
                    bias = bias_sb_get(p + st)
                    xt = xt_live[("kv", b, st, tt)]
                    w = w_sb_get(p + st)
                    if half == 0:
                        acc_live["kv"] = ps.tile([128, NT], F32, tag="acc",
                                                 bufs=1, name="acc")
                    acc = acc_live["kv"]
                    for kt in range(half * 4, half * 4 + 4):
                        nc.tensor.matmul(
                            acc[:], w[:, kt, :], xt[:, kt, :],
                            start=(kt == 0), stop=(kt == DKT - 1),
                        )
                    if half == 1:
                        dst = {"k": kt_sb[(b, st)], "v": vt_sb[(b, st)]}[p]
                        nc.vector.tensor_scalar_add(
                            dst[:, tt * NT:(tt + 1) * NT], acc[:], bias)
                return (4.0, fn,
                        ("P", b, st, p, tt) if half == 1 else None)

            def u_proj_q(b, st, tt):
                # one full Q projection group, atomic, on the "pwo" ring
                # (shared with Wo/transpose units, which are also atomic).
                def fn():
                    bias = bias_sb_get("q" + st)
                    xt = xt_live[("q", b, st, tt)]
                    w = w_sb_get("q" + st)
                    acc = ps.tile([128, NT], F32, tag="pwo", bufs=1,
                                  name="accq")
                    for kt in range(DKT):
                        nc.tensor.matmul(
                            acc[:], w[:, kt, :], xt[:, kt, :],
                            start=(kt == 0), stop=(kt == DKT - 1),
                        )
                    nc.vector.tensor_scalar_add(
                        qt_sb[(b, st)][:, tt * NT:(tt + 1) * NT], acc[:],
                        bias)
                return (8.0, fn, ("P", b, st, "q", tt))

            def u_vaug_init(b, st):
                def fn():
                    Vaug = vaug_sb[(b, st)]
                    nc.vector.memset(Vaug[:, :, 64:65], 1.0)
                    nc.vector.memset(Vaug[:, :, 129:130], 1.0)
                return (0.0, fn, None)

            def u_tr(b, st, k16):
                # PE transpose of one 128-key block of VT into V_aug
                def fn():
                    VT = vt_sb[(b, st)]
                    Vaug = vaug_sb[(b, st)]
                    trp = ps.tile([128, 128], BF16, tag="pwo", bufs=1,
                                  name="trp")
                    nc.tensor.transpose(
                        trp[:], VT[:, k16 * 128:(k16 + 1) * 128], ident[:])
                    nc.vector.tensor_copy(Vaug[:, k16, 0:64], trp[:, 0:64])
                    nc.vector.tensor_copy(Vaug[:, k16, 65:129], trp[:, 64:128])
                return (0.7, fn, ("T", b, st, k16))

            def queue_kv_block(b, st, skip_dma0=False):
                """K/V projections + transposes for (b, st), k16-ordered."""
                get_proj_tiles(b, st)
                fq_kv.append(u_vaug_init(b, st))
                # prefetch depth 3: post three token tiles' loads up
                # front (each ~13us in flight; consumed every ~4.6us)
                if not skip_dma0:
                    fq_kv.append(u_dma_x("kv", b, st, 0))
                fq_kv.append(u_dma_x("kv", b, st, 1))
                fq_kv.append(u_dma_x("kv", b, st, 2))
                for tt in range(NTT):
                    if tt == 1:
                        fq_kv.append(u_dma_x("kv", b, st, 3))
                    for p in ("k", "v"):
                        fq_kv.append(u_proj_kv_half(b, st, tt, p, 0))
                        fq_kv.append(u_proj_kv_half(b, st, tt, p, 1))
                    for k16 in range(tt * 4, tt * 4 + 4):
                        fq_kv.append(u_tr(b, st, k16))

            def queue_q_block(b, st, skip_dma0=False):
                # single-buffered xq ring: each tt's DMA must follow the
                # previous tt's projection (WAR), and Q is consumed only
                # once per query tile (~18us apart) so depth 1 suffices.
                get_proj_tiles(b, st)
                if not skip_dma0:
                    fq_q.append(u_dma_x("q", b, st, 0))
                for tt in range(NTT):
                    fq_q.append(u_proj_q(b, st, tt))
                    if tt + 1 < NTT:
                        fq_q.append(u_dma_x("q", b, st, tt + 1))

            # ---- Wo output-projection filler units ----------------------
            _wo_alt = [0]

            def u_wo(wo, mt, AT_, qsl_, od_, b_):
                def fn():
                    # Once the projection queues are drained the "acc"
                    # bank is free for good; alternating the two banks
                    # lets tail Wo matmuls double-buffer instead of
                    # stalling on each DVE drain.
                    if not (fq_kv or fq_q):
                        _wo_alt[0] ^= 1
                        tag = ("pwo", "acc")[_wo_alt[0]]
                    else:
                        tag = "pwo"
                    po = ps.tile([128, NQ], F32, tag=tag, bufs=1, name="po")
                    nc.tensor.matmul(po[:], wo[:, mt, :], AT_[:, qsl_],
                                     start=True, stop=True)
                    ot = opool.tile([128, NQ], BF16, tag="ot", name="ot")
                    nc.vector.tensor_copy(ot[:], po[:])
                    nc.sync.dma_start(
                        od_[b_, mt * 128:(mt + 1) * 128, qsl_], ot[:])
                return (1.0, fn, None)

            # ---- attention phase ----------------------------------------
            def attention(b, qst, kvst, ost):
                QT = qt_sb[(b, qst)]
                KTt = kt_sb[(b, kvst)]
                Vaug = vaug_sb[(b, kvst)]
                AT = projpool.tile([JC, S], BF16, tag=f"AT_{ost}", bufs=2)
                wo = wo_sb_get(ost)
                for qt in range(NQT):
                    qsl = slice(qt * NQ, (qt + 1) * NQ)
                    ensure(("P", b, qst, "q", qt))
                    pv0 = ps.tile([65, NQ], F32, tag="pv0")
                    pv1 = ps.tile([65, NQ], F32, tag="pv1")
                    es = [None] * NKT

                    def stage_s(k16, _es=es, _K=KTt, _Q=QT, _q=qsl):
                        ensure(("P", b, kvst, "k", k16 // 4))
                        ksl = slice(k16 * 128, (k16 + 1) * 128)
                        sp = ps.tile([128, 2, NQ], F32, tag="spair", bufs=2)
                        nc.tensor.matmul(sp[:, 0, :], _K[0:64, ksl],
                                         _Q[0:64, _q], start=True, stop=True)
                        nc.tensor.matmul(sp[:, 1, :], _K[64:128, ksl],
                                         _Q[64:128, _q], start=True, stop=True)
                        e01 = epool.tile([128, 2, NQ], BF16, tag="e01")
                        nc.scalar.activation(e01[:], sp[:], EXP, scale=SCALE)
                        _es[k16] = e01

                    def stage_pv(k16, _es=es, _V=Vaug, _pv0=pv0, _pv1=pv1):
                        ensure(("T", b, kvst, k16))
                        e01 = _es[k16]
                        nc.tensor.matmul(_pv0[:], _V[:, k16, 0:65],
                                         e01[:, 0, :],
                                         start=(k16 == 0), stop=(k16 == NKT - 1))
                        nc.tensor.matmul(_pv1[:], _V[:, k16, 65:130],
                                         e01[:, 1, :],
                                         start=(k16 == 0), stop=(k16 == NKT - 1))
                        _es[k16] = None

                    for k16 in range(NKT):
                        add_credit(POPS_PER_TILE)
                        stage_s(k16)
                        if k16 >= PV_LAG:
                            stage_pv(k16 - PV_LAG)
                        pop_filler()
                    for k16 in range(NKT - PV_LAG, NKT):
                        add_credit(1.0)
                        stage_pv(k16)
                        pop_filler()
                    # normalize: A^T = PV[:64] * bcast(1 / PV[64]).
                    # Denominators bounce PSUM->SBUF (raf can't read PSUM
                    # on HW), one fast-reciprocal pass, one combined
                    # partition broadcast for both heads.
                    # Interleave the two heads' chains so head1's gpsimd
                    # broadcast overlaps head0's DVE multiply.
                    den = spool.tile([1, 2, NQ], F32, tag="den")
                    rec = spool.tile([1, 2, NQ], F32, tag="rec")
                    rb = spool.tile([64, 2, NQ], F32, tag="rb")
                    nc.vector.tensor_copy(den[0:1, 0, :], pv0[64:65, :])
                    nc.vector.reciprocal_approx_fast(rec[0:1, 0, :],
                                                     den[0:1, 0, :])
                    nc.gpsimd.partition_broadcast(rb[:, 0, :], rec[0:1, 0, :])
                    nc.vector.tensor_copy(den[0:1, 1, :], pv1[64:65, :])
                    nc.vector.reciprocal_approx_fast(rec[0:1, 1, :],
                                                     den[0:1, 1, :])
                    nc.vector.tensor_mul(AT[0:64, qsl], pv0[0:64, :],
                                         rb[:, 0, :])
                    nc.gpsimd.partition_broadcast(rb[:, 1, :], rec[0:1, 1, :])
                    nc.vector.tensor_mul(AT[64:128, qsl], pv1[0:64, :],
                                         rb[:, 1, :])
                    for mt in range(NMT):
                        fq_wo.append(u_wo(wo, mt, AT, qsl, o_dram[ost], b))

            # ---- master sequence ----------------------------------------
            # Kick the gating DMAs (first input tiles, first weights)
            # immediately so the first scores matmul isn't waiting on a
            # cold queue.
            # Startup postings fan out across three DGE queues: kv input
            # on qSP, first q-side input on the (still idle) qACT, and
            # weights/biases on qDVE.
            get_proj_tiles(0, "i")
            get_proj_tiles(0, "v")
            u_dma_x("kv", 0, "i", 0)[1]()
            u_dma_x("q", 0, "v", 0, eng=nc.scalar)[1]()
            for wk in ("ki", "vi", "qv"):
                w_sb_get(wk, eng=nc.scalar)
            load_biases()
            pe_warm_spin()

            queue_kv_block(0, "i", skip_dma0=True)
            queue_kv_block(0, "v")
            queue_q_block(0, "v", skip_dma0=True)
            queue_q_block(0, "i")

            attention(0, "v", "i", "i")
            queue_kv_block(1, "i")
            queue_q_block(1, "v")
            attention(0, "i", "v", "v")
            queue_kv_block(1, "v")
            queue_q_block(1, "i")
            attention(1, "v", "i", "i")
            attention(1, "i", "v", "v")

            flush_filler()

    nc.compile()
    return nc


_NC_CACHE = None


def _get_nc():
    global _NC_CACHE
    if _NC_CACHE is None:
        _NC_CACHE = build_kernel()
    return _NC_CACHE


def kernel(vis, inf, W_q_vis, b_q_vis, W_k_vis, b_k_vis, W_v_vis, b_v_vis,
           W_q_inf, b_q_inf, W_k_inf, b_k_inf, W_v_inf, b_v_inf,
           W_o_vis, b_o_vis, W_o_inf, b_o_inf):
    from concourse.bass_utils import run_bass_kernel_spmd

    nc = _get_nc()
    bf = ml_dtypes.bfloat16
    visT = np.ascontiguousarray(np.asarray(vis).transpose(0, 2, 1)).astype(bf)
    infT = np.ascontiguousarray(np.asarray(inf).transpose(0, 2, 1)).astype(bf)

    wq = {"v": np.asarray(W_q_vis), "i": np.asarray(W_q_inf)}
    wk = {"v": np.asarray(W_k_vis), "i": np.asarray(W_k_inf)}
    wv = {"v": np.asarray(W_v_vis), "i": np.asarray(W_v_inf)}
    bq = {"v": np.asarray(b_q_vis), "i": np.asarray(b_q_inf)}
    bk = {"v": np.asarray(b_k_vis), "i": np.asarray(b_k_inf)}
    bv = {"v": np.asarray(b_v_vis), "i": np.asarray(b_v_inf)}
    wo = {"v": np.asarray(W_o_vis), "i": np.asarray(W_o_inf)}

    in_maps = []
    for c in range(N_CORES):
        sl = slice(c * JC, (c + 1) * JC)
        m = {"visT": visT, "infT": infT}
        for st in ("v", "i"):
            m[f"w_q{st}"] = np.ascontiguousarray(wq[st][:, sl]).astype(bf)
            m[f"w_k{st}"] = np.ascontiguousarray(wk[st][:, sl]).astype(bf)
            m[f"w_v{st}"] = np.ascontiguousarray(wv[st][:, sl]).astype(bf)
            m[f"b_q{st}"] = np.ascontiguousarray(bq[st][sl]).astype(np.float32)
            m[f"b_k{st}"] = np.ascontiguousarray(bk[st][sl]).astype(np.float32)
            m[f"b_v{st}"] = np.ascontiguousarray(bv[st][sl]).astype(np.float32)
        m["w_ov"] = np.ascontiguousarray(wo["v"][sl, :]).astype(bf)
        m["w_oi"] = np.ascontiguousarray(wo["i"][sl, :]).astype(bf)
        in_maps.append(m)

    res = run_bass_kernel_spmd(nc, in_maps, list(range(N_CORES))).results

    ov = np.zeros((B, D, S), np.float32)
    oi = np.zeros((B, D, S), np.float32)
    for c in range(N_CORES):
        ov += res[c]["o_vis"].astype(np.float32)
        oi += res[c]["o_inf"].astype(np.float32)
    out_vis = ov.transpose(0, 2, 1) + np.asarray(b_o_vis)[None, None, :]
    out_inf = oi.transpose(0, 2, 1) + np.asarray(b_o_inf)[None, None, :]
    return (out_vis.astype(np.float32), out_inf.astype(np.float32))


# revision 61
# speedup vs baseline: 1.0495x; 1.0495x over previous
"""Trainium2 Bass kernel for nn_MultiHeadCrossAttention.

Reference computation (B=2, S=2048, D=1024, H=16, HD=64):
  Qv,Kv,Vv = vis @ W_{q,k,v}_vis + b ; Qi,Ki,Vi = inf @ W_{q,k,v}_inf + b
  out_inf = softmax(Qv Ki^T / 8) Vi @ W_o_inf + b_o_inf
  out_vis = softmax(Qi Kv^T / 8) Vv @ W_o_vis + b_o_vis

Sharding: tensor-parallel over the 16 heads; core c owns heads 2c, 2c+1
(columns 128c:128c+128 of the QKV projections, rows of W_o). Each core
computes a full-shape partial of both outputs; the host sums the 8
partials (the "all-reduce after fc_out") and adds the output biases.

Device dataflow is fully transposed (token dim on the free axis):
  QT/KT/VT[j, t] = W.T @ X^T        (W stationary, X^T moving, 8 K-tiles)
  V = transpose(VT) via PE          (+ ones column -> V_aug [128k, 65])
  S^T[k, q]      = KT.T @ QT        (per head, K=64, row-group packed:
                                     both heads' matmuls run concurrently
                                     in disjoint PE row groups)
  E = exp(0.125 * S^T)              (ScalarE, PSUM -> SBUF bf16)
  PV[hd+1, q]    = V_aug.T @ E      (K=128; row 64 = softmax denominator)
  A^T[j, q]      = PV[:64] * bcast(1/PV[64])
  OUT^T[m, t]    = Wo.T @ A^T       (K=128, 8 m-tiles, bf16 partials out)

Schedule: ONE global software pipeline.  The ScalarE exp stream (256
ACTIVATEs x ~1.05us engine-busy = the largest single-engine load) is the
master sequence; all other PE work is filler popped into PE slack
between attention matmuls.  Filler lives in three queues ordered by
consumption time:
  fq_kv: K/V projections + V transposes (a phase's kv side is consumed
         entirely within its FIRST query tile, so these are urgent),
  fq_q:  Q projections (consumed one tile per query tile — relaxed;
         loads its own copy of the input tiles so the two queues share
         no SBUF ring and can pop independently),
  fq_wo: deferred Wo output-projection tiles (no deadline).
ensure() force-pops a queue up to a dependency watermark before any
attention matmul that consumes it is emitted, which both guarantees
deadlock-freedom (every engine queue's order embeds in one global
topological order) and makes the schedule self-correcting.

PSUM budget (8 banks): scores pair [128,2,512]f32 x2 bufs = 4, PV pair
[65,512]f32 x2 = 2, projection accumulator = 1, Wo-out/transpose = 1.
"""

import sys
from collections import deque

for _p in ("/opt/trn_rl_repo", "/root/.axon_site/_ro/trn_rl_repo"):
    if _p not in sys.path:
        sys.path.append(_p)

import numpy as np
import ml_dtypes

import concourse.bass as bass
import concourse.tile as tile
from concourse import bacc, mybir
from concourse.masks import make_identity

F32 = mybir.dt.float32
BF16 = mybir.dt.bfloat16
EXP = mybir.ActivationFunctionType.Exp

B, S, D, H = 2, 2048, 1024, 16
HD = 64
JC = 128          # head dims per core (2 heads x 64)
N_CORES = 8
NT = 512          # token tile (moving dim) for projections
NQ = 512          # query tile for attention
DKT = D // 128    # 8 contraction tiles for projections
SCALE = 1.0 / np.sqrt(HD)

NTT = S // NT     # 4 token tiles per batch
NQT = S // NQ     # 4 query tiles
NKT = S // 128    # 16 key tiles
NMT = D // 128    # 8 output m-tiles

PV_LAG = 3        # PV trails the scores/exp stream by 3 key tiles
POPS_PER_TILE = 2.6


def build_kernel():
    nc = bacc.Bacc()

    visT = nc.dram_tensor("visT", [B, D, S], BF16, kind="ExternalInput")
    infT = nc.dram_tensor("infT", [B, D, S], BF16, kind="ExternalInput")
    w_in = {}
    b_in = {}
    for st in ("v", "i"):
        for p in ("q", "k", "v"):
            w_in[p + st] = nc.dram_tensor(f"w_{p}{st}", [D, JC], BF16, kind="ExternalInput")
            b_in[p + st] = nc.dram_tensor(f"b_{p}{st}", [JC], F32, kind="ExternalInput")
    w_ov = nc.dram_tensor("w_ov", [JC, D], BF16, kind="ExternalInput")
    w_oi = nc.dram_tensor("w_oi", [JC, D], BF16, kind="ExternalInput")
    # bf16 partials: the host sums them in f32, so the only cost is one
    # rounding of each partial (~3e-4 relative on the summed output).
    o_vis = nc.dram_tensor("o_vis", [B, D, S], BF16, kind="ExternalOutput")
    o_inf = nc.dram_tensor("o_inf", [B, D, S], BF16, kind="ExternalOutput")

    with tile.TileContext(nc) as tc:
        with (
            tc.tile_pool(name="const", bufs=1) as cpool,
            tc.tile_pool(name="wpool", bufs=1) as wpool,
            tc.tile_pool(name="proj", bufs=1) as projpool,
            tc.tile_pool(name="xin", bufs=2) as xpool,
            tc.tile_pool(name="esb", bufs=6) as epool,
            tc.tile_pool(name="small", bufs=2) as spool,
            tc.tile_pool(name="outst", bufs=4) as opool,
            tc.tile_pool(name="ps", bufs=1, space="PSUM") as ps,
        ):
            ident = cpool.tile([128, 128], BF16)
            make_identity(nc, ident[:])

            # Pre-load the exp table set (~2.7us) under the prologue DMAs
            # instead of on the first real scores tile.
            warm = cpool.tile([1, 1], F32, tag="warm", name="warm")
            nc.vector.memset(warm[:], 0.0)
            nc.scalar.activation(warm[:], warm[:], EXP)

            def pe_warm_spin():
                # Dummy matmuls bridging the ~13us wait for the first
                # input DMAs: keeps the HAM activity window tripped so
                # the first real matmuls run at 2.4GHz, not 1.2GHz.
                # Alternate two PSUM banks so the WAW chain still
                # pipelines.
                wa = ps.tile([128, 128], F32, tag="acc", bufs=1,
                             name="warmpa")
                wb = ps.tile([65, 128], F32, tag="pv0", name="warmpb")
                for _ in range(36):
                    nc.tensor.matmul(wa[:], ident[:], ident[:],
                                     start=True, stop=True)
                    nc.tensor.matmul(wb[:], ident[:, 0:65], ident[:],
                                     start=True, stop=True)

            # Weight/bias DMAs emitted lazily at first use.
            _w_tiles, _b_tiles, _wo_tiles = {}, {}, {}

            def w_sb_get(key, eng=None):
                # Prologue weight loads post from the still-idle qACT DGE
                # queue so they run in parallel with qSP's input postings
                # (descriptor postings are the scarce serial resource).
                if key not in _w_tiles:
                    e = eng or nc.sync
                    t = wpool.tile([128, DKT, JC], BF16, tag=f"w_{key}",
                                   name=f"w_{key}")
                    src = w_in[key].rearrange("(kt p) j -> p kt j", p=128)
                    e.dma_start(t[:], src)
                    _w_tiles[key] = t
                return _w_tiles[key]

            # All six QKV biases in one batch: six contiguous [1,128] row
            # DMAs (a [128,1] gather costs ~3.5us of descriptor posting
            # each), one cast, ONE K=6 matmul against I6 to flip them to
            # [128, 6], one copy out.  bias_sb_get returns column slices.
            _BKEYS = ("ki", "vi", "qv", "kv", "vv", "qi")
            _bias_state = {}

            def load_biases():
                rows = cpool.tile([6, JC], F32, tag="brows", name="brows")
                for idx, key in enumerate(_BKEYS):
                    nc.scalar.dma_start(rows[idx:idx + 1, :],
                                        b_in[key][:].unsqueeze(0))
                rows16 = cpool.tile([6, JC], BF16, tag="brows16",
                                    name="brows16")
                nc.vector.tensor_copy(rows16[:], rows[:])
                tp = ps.tile([JC, 6], F32, tag="pwo", bufs=1, name="btp")
                nc.tensor.matmul(tp[:], rows16[:], ident[0:6, 0:6],
                                 start=True, stop=True)
                ball = cpool.tile([JC, 6], F32, tag="ball", name="ball")
                nc.vector.tensor_copy(ball[:], tp[:])
                _bias_state["ball"] = ball

            def bias_sb_get(key):
                idx = _BKEYS.index(key)
                return _bias_state["ball"][:, idx:idx + 1]

            def wo_sb_get(key):
                if key not in _wo_tiles:
                    wd = {"v": w_ov, "i": w_oi}[key]
                    t = wpool.tile([JC, NMT, 128], BF16, tag=f"wo_{key}",
                                   name=f"wo_{key}")
                    nc.sync.dma_start(
                        t[:], wd.rearrange("j (mt m) -> j mt m", m=128))
                    _wo_tiles[key] = t
                return _wo_tiles[key]

            xT = {"v": visT, "i": infT}
            o_dram = {"v": o_vis, "i": o_inf}

            # ---- per-(b, st) projection output tiles --------------------
            # Each tag ring has bufs=2 and exactly two allocations (b=0,
            # b=1), so the two batches never alias.
            qt_sb, kt_sb, vt_sb, vaug_sb = {}, {}, {}, {}

            def get_proj_tiles(b, st):
                key = (b, st)
                if key not in qt_sb:
                    qt_sb[key] = projpool.tile([JC, S], BF16, tag=f"QT_{st}",
                                               bufs=2, name=f"QT_{st}{b}")
                    kt_sb[key] = projpool.tile([JC, S], BF16, tag=f"KT_{st}",
                                               bufs=2, name=f"KT_{st}{b}")
                    vt_sb[key] = projpool.tile([JC, S], BF16, tag=f"VT_{st}",
                                               bufs=2, name=f"VT_{st}{b}")
                    vaug_sb[key] = projpool.tile([128, NKT, 130], BF16,
                                                 tag=f"Vaug_{st}", bufs=2,
                                                 name=f"Vaug_{st}{b}")

            # ---- filler task queues -------------------------------------
            # Credit-based popping: each attention tile adds
            # POPS_PER_TILE of PE-work credit; popped units (including
            # ensure()-forced ones) consume it, so dependency bursts
            # automatically suppress later discretionary pops.  Units are
            # ATOMIC: a whole PSUM accumulation group lives in one unit,
            # so ring-shared PSUM tags are never interleaved mid-group.
            fq_kv = deque()
            fq_q = deque()
            fq_wo = deque()
            _done = set()
            _credit = [0.0]
            _rr = [0]

            def _pop_one(q):
                w, fn, provides = q.popleft()
                fn()
                if provides is not None:
                    _done.add(provides)
                _credit[0] -= max(w, 0.25)

            def add_credit(n):
                _credit[0] = min(_credit[0] + n, 8.0)

            def pop_filler():
                while _credit[0] > 0 and (fq_kv or fq_q or fq_wo):
                    # Drain the Wo backlog preferentially once it builds,
                    # else it all lands in a serial post-attention tail.
                    if len(fq_wo) > 3:
                        _pop_one(fq_wo)
                        continue
                    for _ in range(3):
                        q = (fq_kv, fq_q, fq_wo)[_rr[0] % 3]
                        _rr[0] += 1
                        if q:
                            _pop_one(q)
                            break

            def ensure(key):
                q = fq_q if (key[0] == "P" and key[3] == "q") else fq_kv
                while key not in _done:
                    assert q, f"dependency {key} not queued"
                    _pop_one(q)

            def flush_filler():
                for q in (fq_kv, fq_q, fq_wo):
                    while q:
                        _pop_one(q)

            # ---- projection filler units --------------------------------
            # side: "kv" tiles feed K/V projections, "q" tiles feed Q.
            # Separate tags so the two queues never share an SBUF ring
            # (the q side re-loads the inputs; DMA bandwidth is cheap).
            xt_live = {}
            acc_live = {}

            def u_dma_x(side, b, st, tt, eng=None):
                # Per-kt 2D transfers: each [128, 512] piece is one clean
                # descriptor posting with its own completion, so the
                # projection matmuls for early kt can start while later
                # pieces are still in flight.
                def fn():
                    e = eng or nc.sync
                    xt = xpool.tile([128, DKT, NT], BF16, tag=f"x{side}_{st}",
                                    bufs=(3 if side == "kv" else 1),
                                    name=f"x{side}_{st}")
                    src = xT[st].rearrange("bb (kt p) t -> bb p kt t", p=128)[
                        b, :, :, tt * NT:(tt + 1) * NT]
                    e.dma_start(xt[:], src)
                    xt_live[(side, b, st, tt)] = xt
                return (0.0, fn, None)

            def u_proj_kv_half(b, st, tt, p, half):
                # half a K/V projection group (4 accumulating matmuls).
                # The "acc" bank is used ONLY by these kv units (plus
                # tail-time Wo units once the proj queues are empty), and
                # both halves sit adjacent in fq_kv, so no other acc
                # allocation can interleave the accumulation group.
                def fn():
                    bias = bias_sb_get(p + st)
                    xt = xt_live[("kv", b, st, tt)]
                    w = w_sb_get(p + st)
                    if half == 0:
                        acc_live["kv"] = ps.tile([128, NT], F32, tag="acc",
                                                 bufs=1, name="acc")
                    acc = acc_live["kv"]
                    for kt in range(half * 4, half * 4 + 4):
                        nc.tensor.matmul(
                            acc[:], w[:, kt, :], xt[:, kt, :],
                            start=(kt == 0), stop=(kt == DKT - 1),
                        )
                    if half == 1:
                        dst = {"k": kt_sb[(b, st)], "v": vt_sb[(b, st)]}[p]
                        nc.vector.tensor_scalar_add(
                            dst[:, tt * NT:(tt + 1) * NT], acc[:], bias)
                return (4.0, fn,
                        ("P", b, st, p, tt) if half == 1 else None)

            def u_proj_q(b, st, tt):
                # one full Q projection group, atomic, on the "pwo" ring
                # (shared with Wo/transpose units, which are also atomic).
                def fn():
                    bias = bias_sb_get("q" + st)
                    xt = xt_live[("q", b, st, tt)]
                    w = w_sb_get("q" + st)
                    acc = ps.tile([128, NT], F32, tag="pwo", bufs=1,
                                  name="accq")
                    for kt in range(DKT):
                        nc.tensor.matmul(
                            acc[:], w[:, kt, :], xt[:, kt, :],
                            start=(kt == 0), stop=(kt == DKT - 1),
                        )
                    nc.vector.tensor_scalar_add(
                        qt_sb[(b, st)][:, tt * NT:(tt + 1) * NT], acc[:],
                        bias)
                return (8.0, fn, ("P", b, st, "q", tt))

            def u_vaug_init(b, st):
                def fn():
                    Vaug = vaug_sb[(b, st)]
                    nc.vector.memset(Vaug[:, :, 64:65], 1.0)
                    nc.vector.memset(Vaug[:, :, 129:130], 1.0)
                return (0.0, fn, None)

            def u_tr(b, st, k16):
                # PE transpose of one 128-key block of VT into V_aug
                def fn():
                    VT = vt_sb[(b, st)]
                    Vaug = vaug_sb[(b, st)]
                    trp = ps.tile([128, 128], BF16, tag="pwo", bufs=1,
                                  name="trp")
                    nc.tensor.transpose(
                        trp[:], VT[:, k16 * 128:(k16 + 1) * 128], ident[:])
                    nc.vector.tensor_copy(Vaug[:, k16, 0:64], trp[:, 0:64])
                    nc.vector.tensor_copy(Vaug[:, k16, 65:129], trp[:, 64:128])
                return (0.7, fn, ("T", b, st, k16))

            def queue_kv_block(b, st, skip_dma0=False):
                """K/V projections + transposes for (b, st), k16-ordered."""
                get_proj_tiles(b, st)
                fq_kv.append(u_vaug_init(b, st))
                # prefetch depth 3: post three token tiles' loads up
                # front (each ~13us in flight; consumed every ~4.6us)
                if not skip_dma0:
                    fq_kv.append(u_dma_x("kv", b, st, 0))
                fq_kv.append(u_dma_x("kv", b, st, 1))
                fq_kv.append(u_dma_x("kv", b, st, 2))
                for tt in range(NTT):
                    if tt == 1:
                        fq_kv.append(u_dma_x("kv", b, st, 3))
                    for p in ("k", "v"):
                        fq_kv.append(u_proj_kv_half(b, st, tt, p, 0))
                        fq_kv.append(u_proj_kv_half(b, st, tt, p, 1))
                    for k16 in range(tt * 4, tt * 4 + 4):
                        fq_kv.append(u_tr(b, st, k16))

            def queue_q_block(b, st, skip_dma0=False):
                # single-buffered xq ring: each tt's DMA must follow the
                # previous tt's projection (WAR), and Q is consumed only
                # once per query tile (~18us apart) so depth 1 suffices.
                get_proj_tiles(b, st)
                if not skip_dma0:
                    fq_q.append(u_dma_x("q", b, st, 0))
                for tt in range(NTT):
                    fq_q.append(u_proj_q(b, st, tt))
                    if tt + 1 < NTT:
                        fq_q.append(u_dma_x("q", b, st, tt + 1))

            # ---- Wo output-projection filler units ----------------------
            _wo_alt = [0]

            def u_wo(wo, mt, AT_, qsl_, od_, b_):
                def fn():
                    # Once the projection queues are drained the "acc"
                    # bank is free for good; alternating the two banks
                    # lets tail Wo matmuls double-buffer instead of
                    # stalling on each DVE drain.
                    if not (fq_kv or fq_q):
                        _wo_alt[0] ^= 1
                        tag = ("pwo", "acc")[_wo_alt[0]]
                    else:
                        tag = "pwo"
                    po = ps.tile([128, NQ], F32, tag=tag, bufs=1, name="po")
                    nc.tensor.matmul(po[:], wo[:, mt, :], AT_[:, qsl_],
                                     start=True, stop=True)
                    ot = opool.tile([128, NQ], BF16, tag="ot", name="ot")
                    nc.vector.tensor_copy(ot[:], po[:])
                    nc.sync.dma_start(
                        od_[b_, mt * 128:(mt + 1) * 128, qsl_], ot[:])
                return (1.0, fn, None)

            # ---- attention phase ----------------------------------------
            def attention(b, qst, kvst, ost):
                QT = qt_sb[(b, qst)]
                KTt = kt_sb[(b, kvst)]
                Vaug = vaug_sb[(b, kvst)]
                AT = projpool.tile([JC, S], BF16, tag=f"AT_{ost}", bufs=2)
                wo = wo_sb_get(ost)
                for qt in range(NQT):
                    qsl = slice(qt * NQ, (qt + 1) * NQ)
                    ensure(("P", b, qst, "q", qt))
                    pv0 = ps.tile([65, NQ], F32, tag="pv0")
                    pv1 = ps.tile([65, NQ], F32, tag="pv1")
                    es = [None] * NKT

                    def stage_s(k16, _es=es, _K=KTt, _Q=QT, _q=qsl):
                        ensure(("P", b, kvst, "k", k16 // 4))
                        ksl = slice(k16 * 128, (k16 + 1) * 128)
                        sp = ps.tile([128, 2, NQ], F32, tag="spair", bufs=2)
                        nc.tensor.matmul(sp[:, 0, :], _K[0:64, ksl],
                                         _Q[0:64, _q], start=True, stop=True)
                        nc.tensor.matmul(sp[:, 1, :], _K[64:128, ksl],
                                         _Q[64:128, _q], start=True, stop=True)
                        e01 = epool.tile([128, 2, NQ], BF16, tag="e01")
                        nc.scalar.activation(e01[:], sp[:], EXP, scale=SCALE)
                        _es[k16] = e01

                    def stage_pv(k16, _es=es, _V=Vaug, _pv0=pv0, _pv1=pv1):
                        ensure(("T", b, kvst, k16))
                        e01 = _es[k16]
                        nc.tensor.matmul(_pv0[:], _V[:, k16, 0:65],
                                         e01[:, 0, :],
                                         start=(k16 == 0), stop=(k16 == NKT - 1))
                        nc.tensor.matmul(_pv1[:], _V[:, k16, 65:130],
                                         e01[:, 1, :],
                                         start=(k16 == 0), stop=(k16 == NKT - 1))
                        _es[k16] = None

                    for k16 in range(NKT):
                        add_credit(POPS_PER_TILE)
                        stage_s(k16)
                        if k16 >= PV_LAG:
                            stage_pv(k16 - PV_LAG)
                        pop_filler()
                    for k16 in range(NKT - PV_LAG, NKT):
                        add_credit(1.0)
                        stage_pv(k16)
                        pop_filler()
                    # normalize: A^T = PV[:64] * bcast(1 / PV[64]).
                    # Denominators bounce PSUM->SBUF (raf can't read PSUM
                    # on HW), one fast-reciprocal pass, one combined
                    # partition broadcast for both heads.
                    # Interleave the two heads' chains so head1's gpsimd
                    # broadcast overlaps head0's DVE multiply.
                    den = spool.tile([1, 2, NQ], F32, tag="den")
                    rec = spool.tile([1, 2, NQ], F32, tag="rec")
                    rb = spool.tile([64, 2, NQ], F32, tag="rb")
                    nc.vector.tensor_copy(den[0:1, 0, :], pv0[64:65, :])
                    nc.vector.reciprocal_approx_fast(rec[0:1, 0, :],
                                                     den[0:1, 0, :])
                    nc.gpsimd.partition_broadcast(rb[:, 0, :], rec[0:1, 0, :])
                    nc.vector.tensor_copy(den[0:1, 1, :], pv1[64:65, :])
                    nc.vector.reciprocal_approx_fast(rec[0:1, 1, :],
                                                     den[0:1, 1, :])
                    nc.vector.tensor_mul(AT[0:64, qsl], pv0[0:64, :],
                                         rb[:, 0, :])
                    nc.gpsimd.partition_broadcast(rb[:, 1, :], rec[0:1, 1, :])
                    nc.vector.tensor_mul(AT[64:128, qsl], pv1[0:64, :],
                                         rb[:, 1, :])
                    for mt in range(NMT):
                        fq_wo.append(u_wo(wo, mt, AT, qsl, o_dram[ost], b))

            # ---- master sequence ----------------------------------------
            # Kick the gating DMAs (first input tiles, first weights)
            # immediately so the first scores matmul isn't waiting on a
            # cold queue.
            # Startup postings fan out across three DGE queues: kv input
            # on qSP, first q-side input on the (still idle) qACT, and
            # weights/biases on qDVE.
            get_proj_tiles(0, "i")
            get_proj_tiles(0, "v")
            u_dma_x("kv", 0, "i", 0)[1]()
            u_dma_x("q", 0, "v", 0, eng=nc.scalar)[1]()
            for wk in ("ki", "vi", "qv"):
                w_sb_get(wk, eng=nc.scalar)
            load_biases()
            pe_warm_spin()

            queue_kv_block(0, "i", skip_dma0=True)
            queue_kv_block(0, "v")
            queue_q_block(0, "v", skip_dma0=True)
            queue_q_block(0, "i")

            attention(0, "v", "i", "i")
            queue_kv_block(1, "i")
            queue_q_block(1, "v")
            attention(0, "i", "v", "v")
            queue_kv_block(1, "v")
            queue_q_block(1, "i")
            attention(1, "v", "i", "i")
            attention(1, "i", "v", "v")

            flush_filler()

    nc.compile()
    return nc


_NC_CACHE = None


def _get_nc():
    global _NC_CACHE
    if _NC_CACHE is None:
        _NC_CACHE = build_kernel()
    return _NC_CACHE


def kernel(vis, inf, W_q_vis, b_q_vis, W_k_vis, b_k_vis, W_v_vis, b_v_vis,
           W_q_inf, b_q_inf, W_k_inf, b_k_inf, W_v_inf, b_v_inf,
           W_o_vis, b_o_vis, W_o_inf, b_o_inf):
    from concourse.bass_utils import run_bass_kernel_spmd

    nc = _get_nc()
    bf = ml_dtypes.bfloat16
    visT = np.ascontiguousarray(np.asarray(vis).transpose(0, 2, 1)).astype(bf)
    infT = np.ascontiguousarray(np.asarray(inf).transpose(0, 2, 1)).astype(bf)

    wq = {"v": np.asarray(W_q_vis), "i": np.asarray(W_q_inf)}
    wk = {"v": np.asarray(W_k_vis), "i": np.asarray(W_k_inf)}
    wv = {"v": np.asarray(W_v_vis), "i": np.asarray(W_v_inf)}
    bq = {"v": np.asarray(b_q_vis), "i": np.asarray(b_q_inf)}
    bk = {"v": np.asarray(b_k_vis), "i": np.asarray(b_k_inf)}
    bv = {"v": np.asarray(b_v_vis), "i": np.asarray(b_v_inf)}
    wo = {"v": np.asarray(W_o_vis), "i": np.asarray(W_o_inf)}

    in_maps = []
    for c in range(N_CORES):
        sl = slice(c * JC, (c + 1) * JC)
        m = {"visT": visT, "infT": infT}
        for st in ("v", "i"):
            m[f"w_q{st}"] = np.ascontiguousarray(wq[st][:, sl]).astype(bf)
            m[f"w_k{st}"] = np.ascontiguousarray(wk[st][:, sl]).astype(bf)
            m[f"w_v{st}"] = np.ascontiguousarray(wv[st][:, sl]).astype(bf)
            m[f"b_q{st}"] = np.ascontiguousarray(bq[st][sl]).astype(np.float32)
            m[f"b_k{st}"] = np.ascontiguousarray(bk[st][sl]).astype(np.float32)
            m[f"b_v{st}"] = np.ascontiguousarray(bv[st][sl]).astype(np.float32)
        m["w_ov"] = np.ascontiguousarray(wo["v"][sl, :]).astype(bf)
        m["w_oi"] = np.ascontiguousarray(wo["i"][sl, :]).astype(bf)
        in_maps.append(m)

    res = run_bass_kernel_spmd(nc, in_maps, list(range(N_CORES))).results

    ov = np.zeros((B, D, S), np.float32)
    oi = np.zeros((B, D, S), np.float32)
    for c in range(N_CORES):
        ov += res[c]["o_vis"].astype(np.float32)
        oi += res[c]["o_inf"].astype(np.float32)
    out_vis = ov.transpose(0, 2, 1) + np.asarray(b_o_vis)[None, None, :]
    out_inf = oi.transpose(0, 2, 1) + np.asarray(b_o_inf)[None, None, :]
    return (out_vis.astype(np.float32), out_inf.astype(np.float32))


# revision 65
# speedup vs baseline: 1.0928x; 1.0413x over previous
"""Trainium2 Bass kernel for nn_MultiHeadCrossAttention.

Reference computation (B=2, S=2048, D=1024, H=16, HD=64):
  Qv,Kv,Vv = vis @ W_{q,k,v}_vis + b ; Qi,Ki,Vi = inf @ W_{q,k,v}_inf + b
  out_inf = softmax(Qv Ki^T / 8) Vi @ W_o_inf + b_o_inf
  out_vis = softmax(Qi Kv^T / 8) Vv @ W_o_vis + b_o_vis

Sharding: tensor-parallel over the 16 heads; core c owns heads 2c, 2c+1
(columns 128c:128c+128 of the QKV projections, rows of W_o). Each core
computes a full-shape partial of both outputs; the host sums the 8
partials (the "all-reduce after fc_out") and adds the output biases.

Device dataflow is fully transposed (token dim on the free axis):
  QT/KT/VT[j, t] = W.T @ X^T        (W stationary, X^T moving, 8 K-tiles)
  V = transpose(VT) via PE          (+ ones column -> V_aug [128k, 65])
  S^T[k, q]      = KT.T @ QT        (per head, K=64, row-group packed:
                                     both heads' matmuls run concurrently
                                     in disjoint PE row groups)
  E = exp(0.125 * S^T)              (ScalarE, PSUM -> SBUF bf16)
  PV[hd+1, q]    = V_aug.T @ E      (K=128; row 64 = softmax denominator)
  A^T[j, q]      = PV[:64] * bcast(1/PV[64])
  OUT^T[m, t]    = Wo.T @ A^T       (K=128, 8 m-tiles, bf16 partials out)

Schedule: ONE global software pipeline.  The ScalarE exp stream (256
ACTIVATEs x ~1.05us engine-busy = the largest single-engine load) is the
master sequence; all other PE work is filler popped into PE slack
between attention matmuls.  Filler lives in three queues ordered by
consumption time:
  fq_kv: K/V projections + V transposes (a phase's kv side is consumed
         entirely within its FIRST query tile, so these are urgent),
  fq_q:  Q projections (consumed one tile per query tile — relaxed;
         loads its own copy of the input tiles so the two queues share
         no SBUF ring and can pop independently),
  fq_wo: deferred Wo output-projection tiles (no deadline).
ensure() force-pops a queue up to a dependency watermark before any
attention matmul that consumes it is emitted, which both guarantees
deadlock-freedom (every engine queue's order embeds in one global
topological order) and makes the schedule self-correcting.

PSUM budget (8 banks): scores pair [128,2,512]f32 x2 bufs = 4, PV pair
[65,512]f32 x2 = 2, projection accumulator = 1, Wo-out/transpose = 1.
"""

import sys
from collections import deque

for _p in ("/opt/trn_rl_repo", "/root/.axon_site/_ro/trn_rl_repo"):
    if _p not in sys.path:
        sys.path.append(_p)

import numpy as np
import ml_dtypes

import concourse.bass as bass
import concourse.tile as tile
from concourse import bacc, mybir
from concourse.masks import make_identity

F32 = mybir.dt.float32
BF16 = mybir.dt.bfloat16
EXP = mybir.ActivationFunctionType.Exp

B, S, D, H = 2, 2048, 1024, 16
HD = 64
JC = 128          # head dims per core (2 heads x 64)
N_CORES = 8
NT = 512          # token tile (moving dim) for projections
NQ = 512          # query tile for attention
DKT = D // 128    # 8 contraction tiles for projections
SCALE = 1.0 / np.sqrt(HD)

NTT = S // NT     # 4 token tiles per batch
NQT = S // NQ     # 4 query tiles
NKT = S // 128    # 16 key tiles
NMT = D // 128    # 8 output m-tiles

PV_LAG = 3        # PV trails the scores/exp stream by 3 key tiles
POPS_PER_TILE = 2.6


def build_kernel():
    nc = bacc.Bacc()

    visT = nc.dram_tensor("visT", [B, D, S], BF16, kind="ExternalInput")
    infT = nc.dram_tensor("infT", [B, D, S], BF16, kind="ExternalInput")
    w_in = {}
    b_in = {}
    for st in ("v", "i"):
        for p in ("q", "k", "v"):
            w_in[p + st] = nc.dram_tensor(f"w_{p}{st}", [D, JC], BF16, kind="ExternalInput")
            b_in[p + st] = nc.dram_tensor(f"b_{p}{st}", [JC], F32, kind="ExternalInput")
    w_ov = nc.dram_tensor("w_ov", [JC, D], BF16, kind="ExternalInput")
    w_oi = nc.dram_tensor("w_oi", [JC, D], BF16, kind="ExternalInput")
    # bf16 partials: the host sums them in f32, so the only cost is one
    # rounding of each partial (~3e-4 relative on the summed output).
    o_vis = nc.dram_tensor("o_vis", [B, D, S], BF16, kind="ExternalOutput")
    o_inf = nc.dram_tensor("o_inf", [B, D, S], BF16, kind="ExternalOutput")

    with tile.TileContext(nc) as tc:
        with (
            tc.tile_pool(name="const", bufs=1) as cpool,
            tc.tile_pool(name="wpool", bufs=1) as wpool,
            tc.tile_pool(name="proj", bufs=1) as projpool,
            tc.tile_pool(name="xin", bufs=2) as xpool,
            tc.tile_pool(name="esb", bufs=4) as epool,
            tc.tile_pool(name="small", bufs=2) as spool,
            tc.tile_pool(name="outst", bufs=4) as opool,
            tc.tile_pool(name="ps", bufs=1, space="PSUM") as ps,
        ):
            ident = cpool.tile([128, 128], BF16)
            make_identity(nc, ident[:])

            # Pre-load the exp table set (~2.7us) under the prologue DMAs
            # instead of on the first real scores tile.
            warm = cpool.tile([1, 1], F32, tag="warm", name="warm")
            nc.vector.memset(warm[:], 0.0)
            nc.scalar.activation(warm[:], warm[:], EXP)

            def pe_warm_spin():
                # Dummy matmuls bridging the ~13us wait for the first
                # input DMAs: keeps the HAM activity window tripped so
                # the first real matmuls run at 2.4GHz, not 1.2GHz.
                # Alternate two PSUM banks so the WAW chain still
                # pipelines.
                wa = ps.tile([128, 128], F32, tag="acc", bufs=1,
                             name="warmpa")
                wb = ps.tile([65, 128], F32, tag="pv0", name="warmpb")
                for _ in range(36):
                    nc.tensor.matmul(wa[:], ident[:], ident[:],
                                     start=True, stop=True)
                    nc.tensor.matmul(wb[:], ident[:, 0:65], ident[:],
                                     start=True, stop=True)

            # Weight/bias DMAs emitted lazily at first use.
            _w_tiles, _b_tiles, _wo_tiles = {}, {}, {}

            def w_sb_get(key, eng=None):
                # Prologue weight loads post from the still-idle qACT DGE
                # queue so they run in parallel with qSP's input postings
                # (descriptor postings are the scarce serial resource).
                if key not in _w_tiles:
                    e = eng or nc.sync
                    t = wpool.tile([128, DKT, JC], BF16, tag=f"w_{key}",
                                   name=f"w_{key}")
                    src = w_in[key].rearrange("(kt p) j -> p kt j", p=128)
                    e.dma_start(t[:], src)
                    _w_tiles[key] = t
                return _w_tiles[key]

            # All six QKV biases in one batch: six contiguous [1,128] row
            # DMAs (a [128,1] gather costs ~3.5us of descriptor posting
            # each), one cast, ONE K=6 matmul against I6 to flip them to
            # [128, 6], one copy out.  bias_sb_get returns column slices.
            _BKEYS = ("ki", "vi", "qv", "kv", "vv", "qi")
            _bias_state = {}

            def load_biases():
                rows = cpool.tile([6, JC], F32, tag="brows", name="brows")
                for idx, key in enumerate(_BKEYS):
                    nc.scalar.dma_start(rows[idx:idx + 1, :],
                                        b_in[key][:].unsqueeze(0))
                rows16 = cpool.tile([6, JC], BF16, tag="brows16",
                                    name="brows16")
                nc.vector.tensor_copy(rows16[:], rows[:])
                tp = ps.tile([JC, 6], F32, tag="pwo", bufs=1, name="btp")
                nc.tensor.matmul(tp[:], rows16[:], ident[0:6, 0:6],
                                 start=True, stop=True)
                ball = cpool.tile([JC, 6], F32, tag="ball", name="ball")
                nc.vector.tensor_copy(ball[:], tp[:])
                _bias_state["ball"] = ball

            def bias_sb_get(key):
                idx = _BKEYS.index(key)
                return _bias_state["ball"][:, idx:idx + 1]

            def wo_sb_get(key):
                if key not in _wo_tiles:
                    wd = {"v": w_ov, "i": w_oi}[key]
                    t = wpool.tile([JC, NMT, 128], BF16, tag=f"wo_{key}",
                                   name=f"wo_{key}")
                    nc.sync.dma_start(
                        t[:], wd.rearrange("j (mt m) -> j mt m", m=128))
                    _wo_tiles[key] = t
                return _wo_tiles[key]

            xT = {"v": visT, "i": infT}
            o_dram = {"v": o_vis, "i": o_inf}

            # ---- per-(b, st) projection output tiles --------------------
            # Each tag ring has bufs=2 and exactly two allocations (b=0,
            # b=1), so the two batches never alias.
            qt_sb, kt_sb, vt_sb, vaug_sb = {}, {}, {}, {}

            def get_proj_tiles(b, st):
                key = (b, st)
                if key not in qt_sb:
                    qt_sb[key] = projpool.tile([JC, S], BF16, tag=f"QT_{st}",
                                               bufs=2, name=f"QT_{st}{b}")
                    kt_sb[key] = projpool.tile([JC, S], BF16, tag=f"KT_{st}",
                                               bufs=2, name=f"KT_{st}{b}")
                    vt_sb[key] = projpool.tile([JC, S], BF16, tag=f"VT_{st}",
                                               bufs=2, name=f"VT_{st}{b}")
                    vaug_sb[key] = projpool.tile([128, NKT, 130], BF16,
                                                 tag=f"Vaug_{st}", bufs=2,
                                                 name=f"Vaug_{st}{b}")

            # ---- filler task queues -------------------------------------
            # Credit-based popping: each attention tile adds
            # POPS_PER_TILE of PE-work credit; popped units (including
            # ensure()-forced ones) consume it, so dependency bursts
            # automatically suppress later discretionary pops.  Units are
            # ATOMIC: a whole PSUM accumulation group lives in one unit,
            # so ring-shared PSUM tags are never interleaved mid-group.
            fq_kv = deque()
            fq_q = deque()
            fq_wo = deque()
            _done = set()
            _credit = [0.0]
            _rr = [0]

            def _pop_one(q):
                w, fn, provides = q.popleft()
                fn()
                if provides is not None:
                    _done.add(provides)
                _credit[0] -= max(w, 0.25)

            def add_credit(n):
                _credit[0] = min(_credit[0] + n, 8.0)

            def pop_filler():
                while _credit[0] > 0 and (fq_kv or fq_q or fq_wo):
                    # Drain the Wo backlog preferentially once it builds,
                    # else it all lands in a serial post-attention tail.
                    if len(fq_wo) > 3:
                        _pop_one(fq_wo)
                        continue
                    for _ in range(3):
                        q = (fq_kv, fq_q, fq_wo)[_rr[0] % 3]
                        _rr[0] += 1
                        if q:
                            _pop_one(q)
                            break

            def ensure(key):
                q = fq_q if (key[0] == "P" and key[3] == "q") else fq_kv
                while key not in _done:
                    assert q, f"dependency {key} not queued"
                    _pop_one(q)

            def flush_filler():
                for q in (fq_kv, fq_q, fq_wo):
                    while q:
                        _pop_one(q)

            # ---- projection filler units --------------------------------
            # side: "kv" tiles feed K/V projections, "q" tiles feed Q.
            # Separate tags so the two queues never share an SBUF ring
            # (the q side re-loads the inputs; DMA bandwidth is cheap).
            xt_live = {}
            acc_live = {}

            def u_dma_x(side, b, st, tt, eng=None):
                # Per-kt 2D transfers: each [128, 512] piece is one clean
                # descriptor posting with its own completion, so the
                # projection matmuls for early kt can start while later
                # pieces are still in flight.
                def fn():
                    e = eng or nc.sync
                    xt = xpool.tile([128, DKT, NT], BF16, tag=f"x{side}_{st}",
                                    bufs=(3 if side == "kv" else 2),
                                    name=f"x{side}_{st}")
                    src = xT[st].rearrange("bb (kt p) t -> bb p kt t", p=128)[
                        b, :, :, tt * NT:(tt + 1) * NT]
                    e.dma_start(xt[:], src)
                    xt_live[(side, b, st, tt)] = xt
                return (0.0, fn, None)

            def u_proj_kv_half(b, st, tt, p, half):
                # half a K/V projection group (4 accumulating matmuls).
                # The "acc" bank is used ONLY by these kv units (plus
                # tail-time Wo units once the proj queues are empty), and
                # both halves sit adjacent in fq_kv, so no other acc
                # allocation can interleave the accumulation group.
                def fn():
                    bias = bias_sb_get(p + st)
                    xt = xt_live[("kv", b, st, tt)]
                    w = w_sb_get(p + st)
                    if half == 0:
                        acc_live["kv"] = ps.tile([128, NT], F32, tag="acc",
                                                 bufs=1, name="acc")
                    acc = acc_live["kv"]
                    for kt in range(half * 4, half * 4 + 4):
                        nc.tensor.matmul(
                            acc[:], w[:, kt, :], xt[:, kt, :],
                            start=(kt == 0), stop=(kt == DKT - 1),
                        )
                    if half == 1:
                        dst = {"k": kt_sb[(b, st)], "v": vt_sb[(b, st)]}[p]
                        nc.vector.tensor_scalar_add(
                            dst[:, tt * NT:(tt + 1) * NT], acc[:], bias)
                return (4.0, fn,
                        ("P", b, st, p, tt) if half == 1 else None)

            def u_proj_q(b, st, tt):
                # one full Q projection group, atomic, on the "pwo" ring
                # (shared with Wo/transpose units, which are also atomic).
                def fn():
                    bias = bias_sb_get("q" + st)
                    xt = xt_live[("q", b, st, tt)]
                    w = w_sb_get("q" + st)
                    acc = ps.tile([128, NT], F32, tag="pwo", bufs=1,
                                  name="accq")
                    for kt in range(DKT):
                        nc.tensor.matmul(
                            acc[:], w[:, kt, :], xt[:, kt, :],
                            start=(kt == 0), stop=(kt == DKT - 1),
                        )
                    nc.vector.tensor_scalar_add(
                        qt_sb[(b, st)][:, tt * NT:(tt + 1) * NT], acc[:],
                        bias)
                return (8.0, fn, ("P", b, st, "q", tt))

            def u_vaug_init(b, st):
                def fn():
                    Vaug = vaug_sb[(b, st)]
                    nc.vector.memset(Vaug[:, :, 64:65], 1.0)
                    nc.vector.memset(Vaug[:, :, 129:130], 1.0)
                return (0.0, fn, None)

            def u_tr(b, st, k16):
                # PE transpose of one 128-key block of VT into V_aug
                def fn():
                    VT = vt_sb[(b, st)]
                    Vaug = vaug_sb[(b, st)]
                    trp = ps.tile([128, 128], BF16, tag="pwo", bufs=1,
                                  name="trp")
                    nc.tensor.transpose(
                        trp[:], VT[:, k16 * 128:(k16 + 1) * 128], ident[:])
                    nc.vector.tensor_copy(Vaug[:, k16, 0:64], trp[:, 0:64])
                    nc.vector.tensor_copy(Vaug[:, k16, 65:129], trp[:, 64:128])
                return (0.7, fn, ("T", b, st, k16))

            def queue_kv_block(b, st, skip_dma0=False):
                """K/V projections + transposes for (b, st), k16-ordered."""
                get_proj_tiles(b, st)
                fq_kv.append(u_vaug_init(b, st))
                # prefetch depth 3: post three token tiles' loads up
                # front (each ~13us in flight; consumed every ~4.6us)
                if not skip_dma0:
                    fq_kv.append(u_dma_x("kv", b, st, 0))
                fq_kv.append(u_dma_x("kv", b, st, 1))
                fq_kv.append(u_dma_x("kv", b, st, 2))
                for tt in range(NTT):
                    if tt == 1:
                        fq_kv.append(u_dma_x("kv", b, st, 3))
                    for p in ("k", "v"):
                        fq_kv.append(u_proj_kv_half(b, st, tt, p, 0))
                        fq_kv.append(u_proj_kv_half(b, st, tt, p, 1))
                    for k16 in range(tt * 4, tt * 4 + 4):
                        fq_kv.append(u_tr(b, st, k16))

            def queue_q_block(b, st, skip_dma0=False):
                # single-buffered xq ring: each tt's DMA must follow the
                # previous tt's projection (WAR), and Q is consumed only
                # once per query tile (~18us apart) so depth 1 suffices.
                get_proj_tiles(b, st)
                if not skip_dma0:
                    fq_q.append(u_dma_x("q", b, st, 0))
                for tt in range(NTT):
                    fq_q.append(u_proj_q(b, st, tt))
                    if tt + 1 < NTT:
                        fq_q.append(u_dma_x("q", b, st, tt + 1))

            # ---- Wo output-projection filler units ----------------------
            _wo_alt = [0]

            def u_wo(wo, mt, AT_, qsl_, od_, b_):
                def fn():
                    # Once the projection queues are drained the "acc"
                    # bank is free for good; alternating the two banks
                    # lets tail Wo matmuls double-buffer instead of
                    # stalling on each DVE drain.
                    if not (fq_kv or fq_q):
                        _wo_alt[0] ^= 1
                        tag = ("pwo", "acc")[_wo_alt[0]]
                    else:
                        tag = "pwo"
                    po = ps.tile([128, NQ], F32, tag=tag, bufs=1, name="po")
                    nc.tensor.matmul(po[:], wo[:, mt, :], AT_[:, qsl_],
                                     start=True, stop=True)
                    ot = opool.tile([128, NQ], BF16, tag="ot", name="ot")
                    nc.vector.tensor_copy(ot[:], po[:])
                    nc.sync.dma_start(
                        od_[b_, mt * 128:(mt + 1) * 128, qsl_], ot[:])
                return (1.0, fn, None)

            # ---- attention phase ----------------------------------------
            def attention(b, qst, kvst, ost):
                QT = qt_sb[(b, qst)]
                KTt = kt_sb[(b, kvst)]
                Vaug = vaug_sb[(b, kvst)]
                AT = projpool.tile([JC, S], BF16, tag=f"AT_{ost}", bufs=2)
                wo = wo_sb_get(ost)
                for qt in range(NQT):
                    qsl = slice(qt * NQ, (qt + 1) * NQ)
                    ensure(("P", b, qst, "q", qt))
                    pv0 = ps.tile([65, NQ], F32, tag="pv0")
                    pv1 = ps.tile([65, NQ], F32, tag="pv1")
                    es = [None] * NKT

                    def stage_s(k16, _es=es, _K=KTt, _Q=QT, _q=qsl):
                        ensure(("P", b, kvst, "k", k16 // 4))
                        ksl = slice(k16 * 128, (k16 + 1) * 128)
                        sp = ps.tile([128, 2, NQ], F32, tag="spair", bufs=2)
                        nc.tensor.matmul(sp[:, 0, :], _K[0:64, ksl],
                                         _Q[0:64, _q], start=True, stop=True)
                        nc.tensor.matmul(sp[:, 1, :], _K[64:128, ksl],
                                         _Q[64:128, _q], start=True, stop=True)
                        e01 = epool.tile([128, 2, NQ], BF16, tag="e01")
                        nc.scalar.activation(e01[:], sp[:], EXP, scale=SCALE)
                        _es[k16] = e01

                    def stage_pv(k16, _es=es, _V=Vaug, _pv0=pv0, _pv1=pv1):
                        ensure(("T", b, kvst, k16))
                        e01 = _es[k16]
                        nc.tensor.matmul(_pv0[:], _V[:, k16, 0:65],
                                         e01[:, 0, :],
                                         start=(k16 == 0), stop=(k16 == NKT - 1))
                        nc.tensor.matmul(_pv1[:], _V[:, k16, 65:130],
                                         e01[:, 1, :],
                                         start=(k16 == 0), stop=(k16 == NKT - 1))
                        _es[k16] = None

                    for k16 in range(NKT):
                        add_credit(POPS_PER_TILE)
                        stage_s(k16)
                        if k16 >= PV_LAG:
                            stage_pv(k16 - PV_LAG)
                        pop_filler()
                    for k16 in range(NKT - PV_LAG, NKT):
                        add_credit(1.0)
                        stage_pv(k16)
                        pop_filler()
                    # normalize: A^T = PV[:64] * bcast(1 / PV[64]).
                    # Denominators bounce PSUM->SBUF (raf can't read PSUM
                    # on HW), one fast-reciprocal pass, one combined
                    # partition broadcast for both heads.
                    # Interleave the two heads' chains so head1's gpsimd
                    # broadcast overlaps head0's DVE multiply.
                    den = spool.tile([1, 2, NQ], F32, tag="den", bufs=1)
                    rec = spool.tile([1, 2, NQ], F32, tag="rec", bufs=1)
                    rb = spool.tile([64, 2, NQ], F32, tag="rb", bufs=1)
                    nc.vector.tensor_copy(den[0:1, 0, :], pv0[64:65, :])
                    nc.vector.reciprocal_approx_fast(rec[0:1, 0, :],
                                                     den[0:1, 0, :])
                    nc.gpsimd.partition_broadcast(rb[:, 0, :], rec[0:1, 0, :])
                    nc.vector.tensor_copy(den[0:1, 1, :], pv1[64:65, :])
                    nc.vector.reciprocal_approx_fast(rec[0:1, 1, :],
                                                     den[0:1, 1, :])
                    nc.vector.tensor_mul(AT[0:64, qsl], pv0[0:64, :],
                                         rb[:, 0, :])
                    nc.gpsimd.partition_broadcast(rb[:, 1, :], rec[0:1, 1, :])
                    nc.vector.tensor_mul(AT[64:128, qsl], pv1[0:64, :],
                                         rb[:, 1, :])
                    for mt in range(NMT):
                        fq_wo.append(u_wo(wo, mt, AT, qsl, o_dram[ost], b))

            # ---- master sequence ----------------------------------------
            # Kick the gating DMAs (first input tiles, first weights)
            # immediately so the first scores matmul isn't waiting on a
            # cold queue.
            # Startup postings fan out across three DGE queues: kv input
            # on qSP, first q-side input on the (still idle) qACT, and
            # weights/biases on qDVE.
            get_proj_tiles(0, "i")
            get_proj_tiles(0, "v")
            u_dma_x("kv", 0, "i", 0)[1]()          # qSP
            load_biases()                          # qACT (tiny, first)
            u_dma_x("q", 0, "v", 0, eng=nc.scalar)[1]()  # qACT
            for wk in ("ki", "vi", "qv"):
                w_sb_get(wk)                       # qSP, behind xkv
            pe_warm_spin()

            queue_kv_block(0, "i", skip_dma0=True)
            queue_kv_block(0, "v")
            queue_q_block(0, "v", skip_dma0=True)
            queue_q_block(0, "i")

            attention(0, "v", "i", "i")
            queue_kv_block(1, "i")
            queue_q_block(1, "v")
            attention(0, "i", "v", "v")
            queue_kv_block(1, "v")
            queue_q_block(1, "i")
            attention(1, "v", "i", "i")
            attention(1, "i", "v", "v")

            flush_filler()

    nc.compile()
    return nc


_NC_CACHE = None


def _get_nc():
    global _NC_CACHE
    if _NC_CACHE is None:
        _NC_CACHE = build_kernel()
    return _NC_CACHE


def kernel(vis, inf, W_q_vis, b_q_vis, W_k_vis, b_k_vis, W_v_vis, b_v_vis,
           W_q_inf, b_q_inf, W_k_inf, b_k_inf, W_v_inf, b_v_inf,
           W_o_vis, b_o_vis, W_o_inf, b_o_inf):
    from concourse.bass_utils import run_bass_kernel_spmd

    nc = _get_nc()
    bf = ml_dtypes.bfloat16
    visT = np.ascontiguousarray(np.asarray(vis).transpose(0, 2, 1)).astype(bf)
    infT = np.ascontiguousarray(np.asarray(inf).transpose(0, 2, 1)).astype(bf)

    wq = {"v": np.asarray(W_q_vis), "i": np.asarray(W_q_inf)}
    wk = {"v": np.asarray(W_k_vis), "i": np.asarray(W_k_inf)}
    wv = {"v": np.asarray(W_v_vis), "i": np.asarray(W_v_inf)}
    bq = {"v": np.asarray(b_q_vis), "i": np.asarray(b_q_inf)}
    bk = {"v": np.asarray(b_k_vis), "i": np.asarray(b_k_inf)}
    bv = {"v": np.asarray(b_v_vis), "i": np.asarray(b_v_inf)}
    wo = {"v": np.asarray(W_o_vis), "i": np.asarray(W_o_inf)}

    in_maps = []
    for c in range(N_CORES):
        sl = slice(c * JC, (c + 1) * JC)
        m = {"visT": visT, "infT": infT}
        for st in ("v", "i"):
            m[f"w_q{st}"] = np.ascontiguousarray(wq[st][:, sl]).astype(bf)
            m[f"w_k{st}"] = np.ascontiguousarray(wk[st][:, sl]).astype(bf)
            m[f"w_v{st}"] = np.ascontiguousarray(wv[st][:, sl]).astype(bf)
            m[f"b_q{st}"] = np.ascontiguousarray(bq[st][sl]).astype(np.float32)
            m[f"b_k{st}"] = np.ascontiguousarray(bk[st][sl]).astype(np.float32)
            m[f"b_v{st}"] = np.ascontiguousarray(bv[st][sl]).astype(np.float32)
        m["w_ov"] = np.ascontiguousarray(wo["v"][sl, :]).astype(bf)
        m["w_oi"] = np.ascontiguousarray(wo["i"][sl, :]).astype(bf)
        in_maps.append(m)

    res = run_bass_kernel_spmd(nc, in_maps, list(range(N_CORES))).results

    ov = np.zeros((B, D, S), np.float32)
    oi = np.zeros((B, D, S), np.float32)
    for c in range(N_CORES):
        ov += res[c]["o_vis"].astype(np.float32)
        oi += res[c]["o_inf"].astype(np.float32)
    out_vis = ov.transpose(0, 2, 1) + np.asarray(b_o_vis)[None, None, :]
    out_inf = oi.transpose(0, 2, 1) + np.asarray(b_o_inf)[None, None, :]
    return (out_vis.astype(np.float32), out_inf.astype(np.float32))


# revision 66
# speedup vs baseline: 1.1000x; 1.0066x over previous
"""Trainium2 Bass kernel for nn_MultiHeadCrossAttention.

Reference computation (B=2, S=2048, D=1024, H=16, HD=64):
  Qv,Kv,Vv = vis @ W_{q,k,v}_vis + b ; Qi,Ki,Vi = inf @ W_{q,k,v}_inf + b
  out_inf = softmax(Qv Ki^T / 8) Vi @ W_o_inf + b_o_inf
  out_vis = softmax(Qi Kv^T / 8) Vv @ W_o_vis + b_o_vis

Sharding: tensor-parallel over the 16 heads; core c owns heads 2c, 2c+1
(columns 128c:128c+128 of the QKV projections, rows of W_o). Each core
computes a full-shape partial of both outputs; the host sums the 8
partials (the "all-reduce after fc_out") and adds the output biases.

Device dataflow is fully transposed (token dim on the free axis):
  QT/KT/VT[j, t] = W.T @ X^T        (W stationary, X^T moving, 8 K-tiles)
  V = transpose(VT) via PE          (+ ones column -> V_aug [128k, 65])
  S^T[k, q]      = KT.T @ QT        (per head, K=64, row-group packed:
                                     both heads' matmuls run concurrently
                                     in disjoint PE row groups)
  E = exp(0.125 * S^T)              (ScalarE, PSUM -> SBUF bf16)
  PV[hd+1, q]    = V_aug.T @ E      (K=128; row 64 = softmax denominator)
  A^T[j, q]      = PV[:64] * bcast(1/PV[64])
  OUT^T[m, t]    = Wo.T @ A^T       (K=128, 8 m-tiles, bf16 partials out)

Schedule: ONE global software pipeline.  The ScalarE exp stream (256
ACTIVATEs x ~1.05us engine-busy = the largest single-engine load) is the
master sequence; all other PE work is filler popped into PE slack
between attention matmuls.  Filler lives in three queues ordered by
consumption time:
  fq_kv: K/V projections + V transposes (a phase's kv side is consumed
         entirely within its FIRST query tile, so these are urgent),
  fq_q:  Q projections (consumed one tile per query tile — relaxed;
         loads its own copy of the input tiles so the two queues share
         no SBUF ring and can pop independently),
  fq_wo: deferred Wo output-projection tiles (no deadline).
ensure() force-pops a queue up to a dependency watermark before any
attention matmul that consumes it is emitted, which both guarantees
deadlock-freedom (every engine queue's order embeds in one global
topological order) and makes the schedule self-correcting.

PSUM budget (8 banks): scores pair [128,2,512]f32 x2 bufs = 4, PV pair
[65,512]f32 x2 = 2, projection accumulator = 1, Wo-out/transpose = 1.
"""

import sys
from collections import deque

for _p in ("/opt/trn_rl_repo", "/root/.axon_site/_ro/trn_rl_repo"):
    if _p not in sys.path:
        sys.path.append(_p)

import numpy as np
import ml_dtypes

import concourse.bass as bass
import concourse.tile as tile
from concourse import bacc, mybir
from concourse.masks import make_identity

F32 = mybir.dt.float32
BF16 = mybir.dt.bfloat16
EXP = mybir.ActivationFunctionType.Exp

B, S, D, H = 2, 2048, 1024, 16
HD = 64
JC = 128          # head dims per core (2 heads x 64)
N_CORES = 8
NT = 512          # token tile (moving dim) for projections
NQ = 512          # query tile for attention
DKT = D // 128    # 8 contraction tiles for projections
SCALE = 1.0 / np.sqrt(HD)

NTT = S // NT     # 4 token tiles per batch
NQT = S // NQ     # 4 query tiles
NKT = S // 128    # 16 key tiles
NMT = D // 128    # 8 output m-tiles

PV_LAG = 3        # PV trails the scores/exp stream by 3 key tiles
POPS_PER_TILE = 2.6


def build_kernel():
    nc = bacc.Bacc()

    visT = nc.dram_tensor("visT", [B, D, S], BF16, kind="ExternalInput")
    infT = nc.dram_tensor("infT", [B, D, S], BF16, kind="ExternalInput")
    w_in = {}
    b_in = {}
    for st in ("v", "i"):
        for p in ("q", "k", "v"):
            w_in[p + st] = nc.dram_tensor(f"w_{p}{st}", [D, JC], BF16, kind="ExternalInput")
            b_in[p + st] = nc.dram_tensor(f"b_{p}{st}", [JC], F32, kind="ExternalInput")
    w_ov = nc.dram_tensor("w_ov", [JC, D], BF16, kind="ExternalInput")
    w_oi = nc.dram_tensor("w_oi", [JC, D], BF16, kind="ExternalInput")
    # bf16 partials: the host sums them in f32, so the only cost is one
    # rounding of each partial (~3e-4 relative on the summed output).
    o_vis = nc.dram_tensor("o_vis", [B, D, S], BF16, kind="ExternalOutput")
    o_inf = nc.dram_tensor("o_inf", [B, D, S], BF16, kind="ExternalOutput")

    with tile.TileContext(nc) as tc:
        with (
            tc.tile_pool(name="const", bufs=1) as cpool,
            tc.tile_pool(name="wpool", bufs=1) as wpool,
            tc.tile_pool(name="proj", bufs=1) as projpool,
            tc.tile_pool(name="xin", bufs=2) as xpool,
            tc.tile_pool(name="esb", bufs=4) as epool,
            tc.tile_pool(name="small", bufs=2) as spool,
            tc.tile_pool(name="outst", bufs=4) as opool,
            tc.tile_pool(name="ps", bufs=1, space="PSUM") as ps,
        ):
            ident = cpool.tile([128, 128], BF16)
            make_identity(nc, ident[:])

            # Pre-load the exp table set (~2.7us) under the prologue DMAs
            # instead of on the first real scores tile.
            warm = cpool.tile([1, 1], F32, tag="warm", name="warm")
            nc.vector.memset(warm[:], 0.0)
            nc.scalar.activation(warm[:], warm[:], EXP)

            def pe_warm_spin():
                # Dummy matmuls bridging the ~13us wait for the first
                # input DMAs: keeps the HAM activity window tripped so
                # the first real matmuls run at 2.4GHz, not 1.2GHz.
                # Alternate two PSUM banks so the WAW chain still
                # pipelines.
                wa = ps.tile([128, 128], F32, tag="acc", bufs=1,
                             name="warmpa")
                wb = ps.tile([65, 128], F32, tag="pv0", name="warmpb")
                for _ in range(36):
                    nc.tensor.matmul(wa[:], ident[:], ident[:],
                                     start=True, stop=True)
                    nc.tensor.matmul(wb[:], ident[:, 0:65], ident[:],
                                     start=True, stop=True)

            # Weight/bias DMAs emitted lazily at first use.
            _w_tiles, _b_tiles, _wo_tiles = {}, {}, {}

            def w_sb_get(key, eng=None):
                # Prologue weight loads post from the still-idle qACT DGE
                # queue so they run in parallel with qSP's input postings
                # (descriptor postings are the scarce serial resource).
                if key not in _w_tiles:
                    e = eng or nc.sync
                    t = wpool.tile([128, DKT, JC], BF16, tag=f"w_{key}",
                                   name=f"w_{key}")
                    src = w_in[key].rearrange("(kt p) j -> p kt j", p=128)
                    e.dma_start(t[:], src)
                    _w_tiles[key] = t
                return _w_tiles[key]

            # All six QKV biases in one batch: six contiguous [1,128] row
            # DMAs (a [128,1] gather costs ~3.5us of descriptor posting
            # each), one cast, ONE K=6 matmul against I6 to flip them to
            # [128, 6], one copy out.  bias_sb_get returns column slices.
            _BKEYS = ("ki", "vi", "qv", "kv", "vv", "qi")
            _bias_state = {}

            def load_biases():
                rows = cpool.tile([6, JC], F32, tag="brows", name="brows")
                for idx, key in enumerate(_BKEYS):
                    nc.scalar.dma_start(rows[idx:idx + 1, :],
                                        b_in[key][:].unsqueeze(0))
                rows16 = cpool.tile([6, JC], BF16, tag="brows16",
                                    name="brows16")
                nc.vector.tensor_copy(rows16[:], rows[:])
                tp = ps.tile([JC, 6], F32, tag="pwo", bufs=1, name="btp")
                nc.tensor.matmul(tp[:], rows16[:], ident[0:6, 0:6],
                                 start=True, stop=True)
                ball = cpool.tile([JC, 6], F32, tag="ball", name="ball")
                nc.vector.tensor_copy(ball[:], tp[:])
                _bias_state["ball"] = ball

            def bias_sb_get(key):
                idx = _BKEYS.index(key)
                return _bias_state["ball"][:, idx:idx + 1]

            def wo_sb_get(key):
                if key not in _wo_tiles:
                    wd = {"v": w_ov, "i": w_oi}[key]
                    t = wpool.tile([JC, NMT, 128], BF16, tag=f"wo_{key}",
                                   name=f"wo_{key}")
                    nc.sync.dma_start(
                        t[:], wd.rearrange("j (mt m) -> j mt m", m=128))
                    _wo_tiles[key] = t
                return _wo_tiles[key]

            xT = {"v": visT, "i": infT}
            o_dram = {"v": o_vis, "i": o_inf}

            # ---- per-(b, st) projection output tiles --------------------
            # Each tag ring has bufs=2 and exactly two allocations (b=0,
            # b=1), so the two batches never alias.
            qt_sb, kt_sb, vt_sb, vaug_sb = {}, {}, {}, {}

            def get_proj_tiles(b, st):
                key = (b, st)
                if key not in qt_sb:
                    qt_sb[key] = projpool.tile([JC, S], BF16, tag=f"QT_{st}",
                                               bufs=2, name=f"QT_{st}{b}")
                    kt_sb[key] = projpool.tile([JC, S], BF16, tag=f"KT_{st}",
                                               bufs=2, name=f"KT_{st}{b}")
                    vt_sb[key] = projpool.tile([JC, S], BF16, tag=f"VT_{st}",
                                               bufs=2, name=f"VT_{st}{b}")
                    vaug_sb[key] = projpool.tile([128, NKT, 130], BF16,
                                                 tag=f"Vaug_{st}", bufs=2,
                                                 name=f"Vaug_{st}{b}")

            # ---- filler task queues -------------------------------------
            # Credit-based popping: each attention tile adds
            # POPS_PER_TILE of PE-work credit; popped units (including
            # ensure()-forced ones) consume it, so dependency bursts
            # automatically suppress later discretionary pops.  Units are
            # ATOMIC: a whole PSUM accumulation group lives in one unit,
            # so ring-shared PSUM tags are never interleaved mid-group.
            fq_kv = deque()
            fq_q = deque()
            fq_wo = deque()
            _done = set()
            _credit = [0.0]
            _rr = [0]

            def _pop_one(q):
                w, fn, provides = q.popleft()
                fn()
                if provides is not None:
                    _done.add(provides)
                _credit[0] -= max(w, 0.25)

            def add_credit(n):
                _credit[0] = min(_credit[0] + n, 8.0)

            def pop_filler():
                while _credit[0] > 0 and (fq_kv or fq_q or fq_wo):
                    # Drain the Wo backlog preferentially once it builds,
                    # else it all lands in a serial post-attention tail.
                    if len(fq_wo) > 3:
                        _pop_one(fq_wo)
                        continue
                    for _ in range(3):
                        q = (fq_kv, fq_q, fq_wo)[_rr[0] % 3]
                        _rr[0] += 1
                        if q:
                            _pop_one(q)
                            break

            def ensure(key):
                q = fq_q if (key[0] == "P" and key[3] == "q") else fq_kv
                while key not in _done:
                    assert q, f"dependency {key} not queued"
                    _pop_one(q)

            def flush_filler():
                for q in (fq_kv, fq_q, fq_wo):
                    while q:
                        _pop_one(q)

            # ---- projection filler units --------------------------------
            # side: "kv" tiles feed K/V projections, "q" tiles feed Q.
            # Separate tags so the two queues never share an SBUF ring
            # (the q side re-loads the inputs; DMA bandwidth is cheap).
            xt_live = {}
            acc_live = {}

            def u_dma_x(side, b, st, tt, eng=None):
                # Per-kt 2D transfers: each [128, 512] piece is one clean
                # descriptor posting with its own completion, so the
                # projection matmuls for early kt can start while later
                # pieces are still in flight.
                def fn():
                    e = eng or nc.sync
                    xt = xpool.tile([128, DKT, NT], BF16, tag=f"x{side}_{st}",
                                    bufs=(3 if side == "kv" else 2),
                                    name=f"x{side}_{st}")
                    src = xT[st].rearrange("bb (kt p) t -> bb p kt t", p=128)[
                        b, :, :, tt * NT:(tt + 1) * NT]
                    e.dma_start(xt[:], src)
                    xt_live[(side, b, st, tt)] = xt
                return (0.0, fn, None)

            def u_proj_kv_half(b, st, tt, p, half):
                # half a K/V projection group (4 accumulating matmuls).
                # The "acc" bank is used ONLY by these kv units (plus
                # tail-time Wo units once the proj queues are empty), and
                # both halves sit adjacent in fq_kv, so no other acc
                # allocation can interleave the accumulation group.
                def fn():
                    bias = bias_sb_get(p + st)
                    xt = xt_live[("kv", b, st, tt)]
                    w = w_sb_get(p + st)
                    if half == 0:
                        acc_live["kv"] = ps.tile([128, NT], F32, tag="acc",
                                                 bufs=1, name="acc")
                    acc = acc_live["kv"]
                    for kt in range(half * 4, half * 4 + 4):
                        nc.tensor.matmul(
                            acc[:], w[:, kt, :], xt[:, kt, :],
                            start=(kt == 0), stop=(kt == DKT - 1),
                        )
                    if half == 1:
                        dst = {"k": kt_sb[(b, st)], "v": vt_sb[(b, st)]}[p]
                        nc.vector.tensor_scalar_add(
                            dst[:, tt * NT:(tt + 1) * NT], acc[:], bias)
                return (4.0, fn,
                        ("P", b, st, p, tt) if half == 1 else None)

            def u_proj_q(b, st, tt):
                # one full Q projection group, atomic, on the "pwo" ring
                # (shared with Wo/transpose units, which are also atomic).
                def fn():
                    bias = bias_sb_get("q" + st)
                    xt = xt_live[("q", b, st, tt)]
                    w = w_sb_get("q" + st)
                    acc = ps.tile([128, NT], F32, tag="pwo", bufs=1,
                                  name="accq")
                    for kt in range(DKT):
                        nc.tensor.matmul(
                            acc[:], w[:, kt, :], xt[:, kt, :],
                            start=(kt == 0), stop=(kt == DKT - 1),
                        )
                    nc.vector.tensor_scalar_add(
                        qt_sb[(b, st)][:, tt * NT:(tt + 1) * NT], acc[:],
                        bias)
                return (8.0, fn, ("P", b, st, "q", tt))

            def u_vaug_init(b, st):
                def fn():
                    Vaug = vaug_sb[(b, st)]
                    nc.vector.memset(Vaug[:, :, 64:65], 1.0)
                    nc.vector.memset(Vaug[:, :, 129:130], 1.0)
                return (0.0, fn, None)

            def u_tr(b, st, k16):
                # PE transpose of one 128-key block of VT into V_aug
                def fn():
                    VT = vt_sb[(b, st)]
                    Vaug = vaug_sb[(b, st)]
                    trp = ps.tile([128, 128], BF16, tag="pwo", bufs=1,
                                  name="trp")
                    nc.tensor.transpose(
                        trp[:], VT[:, k16 * 128:(k16 + 1) * 128], ident[:])
                    nc.vector.tensor_copy(Vaug[:, k16, 0:64], trp[:, 0:64])
                    nc.vector.tensor_copy(Vaug[:, k16, 65:129], trp[:, 64:128])
                return (0.7, fn, ("T", b, st, k16))

            def queue_kv_block(b, st, skip_dma0=False):
                """K/V projections + transposes for (b, st), k16-ordered."""
                get_proj_tiles(b, st)
                fq_kv.append(u_vaug_init(b, st))
                # prefetch depth 3: post three token tiles' loads up
                # front (each ~13us in flight; consumed every ~4.6us)
                if not skip_dma0:
                    fq_kv.append(u_dma_x("kv", b, st, 0))
                fq_kv.append(u_dma_x("kv", b, st, 1))
                fq_kv.append(u_dma_x("kv", b, st, 2))
                for tt in range(NTT):
                    if tt == 1:
                        fq_kv.append(u_dma_x("kv", b, st, 3))
                    for p in ("k", "v"):
                        fq_kv.append(u_proj_kv_half(b, st, tt, p, 0))
                        fq_kv.append(u_proj_kv_half(b, st, tt, p, 1))
                    for k16 in range(tt * 4, tt * 4 + 4):
                        fq_kv.append(u_tr(b, st, k16))

            def queue_q_block(b, st, skip_dma0=False):
                # single-buffered xq ring: each tt's DMA must follow the
                # previous tt's projection (WAR), and Q is consumed only
                # once per query tile (~18us apart) so depth 1 suffices.
                get_proj_tiles(b, st)
                if not skip_dma0:
                    fq_q.append(u_dma_x("q", b, st, 0))
                for tt in range(NTT):
                    fq_q.append(u_proj_q(b, st, tt))
                    if tt + 1 < NTT:
                        fq_q.append(u_dma_x("q", b, st, tt + 1))

            # ---- Wo output-projection filler units ----------------------
            _wo_alt = [0]

            def u_wo(wo, mt, AT_, qsl_, od_, b_):
                def fn():
                    # Once the projection queues are drained the "acc"
                    # bank is free for good; alternating the two banks
                    # lets tail Wo matmuls double-buffer instead of
                    # stalling on each DVE drain.
                    if not (fq_kv or fq_q):
                        _wo_alt[0] ^= 1
                        tag = ("pwo", "acc")[_wo_alt[0]]
                    else:
                        tag = "pwo"
                    po = ps.tile([128, NQ], F32, tag=tag, bufs=1, name="po")
                    nc.tensor.matmul(po[:], wo[:, mt, :], AT_[:, qsl_],
                                     start=True, stop=True)
                    ot = opool.tile([128, NQ], BF16, tag="ot", name="ot")
                    nc.vector.tensor_copy(ot[:], po[:])
                    nc.sync.dma_start(
                        od_[b_, mt * 128:(mt + 1) * 128, qsl_], ot[:])
                return (1.0, fn, None)

            # ---- attention phase ----------------------------------------
            def attention(b, qst, kvst, ost):
                QT = qt_sb[(b, qst)]
                KTt = kt_sb[(b, kvst)]
                Vaug = vaug_sb[(b, kvst)]
                AT = projpool.tile([JC, S], BF16, tag=f"AT_{ost}", bufs=2)
                wo = wo_sb_get(ost)
                for qt in range(NQT):
                    qsl = slice(qt * NQ, (qt + 1) * NQ)
                    ensure(("P", b, qst, "q", qt))
                    pv0 = ps.tile([65, NQ], F32, tag="pv0")
                    pv1 = ps.tile([65, NQ], F32, tag="pv1")
                    es = [None] * NKT

                    def stage_s(k16, _es=es, _K=KTt, _Q=QT, _q=qsl):
                        ensure(("P", b, kvst, "k", k16 // 4))
                        ksl = slice(k16 * 128, (k16 + 1) * 128)
                        sp = ps.tile([128, 2, NQ], F32, tag="spair", bufs=2)
                        nc.tensor.matmul(sp[:, 0, :], _K[0:64, ksl],
                                         _Q[0:64, _q], start=True, stop=True)
                        nc.tensor.matmul(sp[:, 1, :], _K[64:128, ksl],
                                         _Q[64:128, _q], start=True, stop=True)
                        e01 = epool.tile([128, 2, NQ], BF16, tag="e01")
                        nc.scalar.activation(e01[:], sp[:], EXP, scale=SCALE)
                        _es[k16] = e01

                    def stage_pv(k16, _es=es, _V=Vaug, _pv0=pv0, _pv1=pv1):
                        ensure(("T", b, kvst, k16))
                        e01 = _es[k16]
                        nc.tensor.matmul(_pv0[:], _V[:, k16, 0:65],
                                         e01[:, 0, :],
                                         start=(k16 == 0), stop=(k16 == NKT - 1))
                        nc.tensor.matmul(_pv1[:], _V[:, k16, 65:130],
                                         e01[:, 1, :],
                                         start=(k16 == 0), stop=(k16 == NKT - 1))
                        _es[k16] = None

                    for k16 in range(NKT):
                        add_credit(POPS_PER_TILE)
                        stage_s(k16)
                        if k16 >= PV_LAG:
                            stage_pv(k16 - PV_LAG)
                        if k16 == 8 and qt + 1 < NQT:
                            # pull the next query tile's Q projection in
                            # mid-stream so the qt boundary doesn't stall
                            # on its atomic 8-matmul group
                            ensure(("P", b, qst, "q", qt + 1))
                        pop_filler()
                    for k16 in range(NKT - PV_LAG, NKT):
                        add_credit(1.0)
                        stage_pv(k16)
                        pop_filler()
                    # normalize: A^T = PV[:64] * bcast(1 / PV[64]).
                    # Denominators bounce PSUM->SBUF (raf can't read PSUM
                    # on HW), one fast-reciprocal pass, one combined
                    # partition broadcast for both heads.
                    # Interleave the two heads' chains so head1's gpsimd
                    # broadcast overlaps head0's DVE multiply.
                    den = spool.tile([1, 2, NQ], F32, tag="den", bufs=1)
                    rec = spool.tile([1, 2, NQ], F32, tag="rec", bufs=1)
                    rb = spool.tile([64, 2, NQ], F32, tag="rb", bufs=1)
                    nc.vector.tensor_copy(den[0:1, 0, :], pv0[64:65, :])
                    nc.vector.reciprocal_approx_fast(rec[0:1, 0, :],
                                                     den[0:1, 0, :])
                    nc.gpsimd.partition_broadcast(rb[:, 0, :], rec[0:1, 0, :])
                    nc.vector.tensor_copy(den[0:1, 1, :], pv1[64:65, :])
                    nc.vector.reciprocal_approx_fast(rec[0:1, 1, :],
                                                     den[0:1, 1, :])
                    nc.vector.tensor_mul(AT[0:64, qsl], pv0[0:64, :],
                                         rb[:, 0, :])
                    nc.gpsimd.partition_broadcast(rb[:, 1, :], rec[0:1, 1, :])
                    nc.vector.tensor_mul(AT[64:128, qsl], pv1[0:64, :],
                                         rb[:, 1, :])
                    for mt in range(NMT):
                        fq_wo.append(u_wo(wo, mt, AT, qsl, o_dram[ost], b))

            # ---- master sequence ----------------------------------------
            # Kick the gating DMAs (first input tiles, first weights)
            # immediately so the first scores matmul isn't waiting on a
            # cold queue.
            # Startup postings fan out across three DGE queues: kv input
            # on qSP, first q-side input on the (still idle) qACT, and
            # weights/biases on qDVE.
            get_proj_tiles(0, "i")
            get_proj_tiles(0, "v")
            u_dma_x("kv", 0, "i", 0)[1]()          # qSP
            load_biases()                          # qACT (tiny, first)
            u_dma_x("q", 0, "v", 0, eng=nc.scalar)[1]()  # qACT
            for wk in ("ki", "vi", "qv"):
                w_sb_get(wk)                       # qSP, behind xkv
            pe_warm_spin()

            queue_kv_block(0, "i", skip_dma0=True)
            queue_kv_block(0, "v")
            queue_q_block(0, "v", skip_dma0=True)
            queue_q_block(0, "i")

            attention(0, "v", "i", "i")
            queue_kv_block(1, "i")
            queue_q_block(1, "v")
            attention(0, "i", "v", "v")
            queue_kv_block(1, "v")
            queue_q_block(1, "i")
            attention(1, "v", "i", "i")
            attention(1, "i", "v", "v")

            flush_filler()

    nc.compile()
    return nc


_NC_CACHE = None


def _get_nc():
    global _NC_CACHE
    if _NC_CACHE is None:
        _NC_CACHE = build_kernel()
    return _NC_CACHE


def kernel(vis, inf, W_q_vis, b_q_vis, W_k_vis, b_k_vis, W_v_vis, b_v_vis,
           W_q_inf, b_q_inf, W_k_inf, b_k_inf, W_v_inf, b_v_inf,
           W_o_vis, b_o_vis, W_o_inf, b_o_inf):
    from concourse.bass_utils import run_bass_kernel_spmd

    nc = _get_nc()
    bf = ml_dtypes.bfloat16
    visT = np.ascontiguousarray(np.asarray(vis).transpose(0, 2, 1)).astype(bf)
    infT = np.ascontiguousarray(np.asarray(inf).transpose(0, 2, 1)).astype(bf)

    wq = {"v": np.asarray(W_q_vis), "i": np.asarray(W_q_inf)}
    wk = {"v": np.asarray(W_k_vis), "i": np.asarray(W_k_inf)}
    wv = {"v": np.asarray(W_v_vis), "i": np.asarray(W_v_inf)}
    bq = {"v": np.asarray(b_q_vis), "i": np.asarray(b_q_inf)}
    bk = {"v": np.asarray(b_k_vis), "i": np.asarray(b_k_inf)}
    bv = {"v": np.asarray(b_v_vis), "i": np.asarray(b_v_inf)}
    wo = {"v": np.asarray(W_o_vis), "i": np.asarray(W_o_inf)}

    in_maps = []
    for c in range(N_CORES):
        sl = slice(c * JC, (c + 1) * JC)
        m = {"visT": visT, "infT": infT}
        for st in ("v", "i"):
            m[f"w_q{st}"] = np.ascontiguousarray(wq[st][:, sl]).astype(bf)
            m[f"w_k{st}"] = np.ascontiguousarray(wk[st][:, sl]).astype(bf)
            m[f"w_v{st}"] = np.ascontiguousarray(wv[st][:, sl]).astype(bf)
            m[f"b_q{st}"] = np.ascontiguousarray(bq[st][sl]).astype(np.float32)
            m[f"b_k{st}"] = np.ascontiguousarray(bk[st][sl]).astype(np.float32)
            m[f"b_v{st}"] = np.ascontiguousarray(bv[st][sl]).astype(np.float32)
        m["w_ov"] = np.ascontiguousarray(wo["v"][sl, :]).astype(bf)
        m["w_oi"] = np.ascontiguousarray(wo["i"][sl, :]).astype(bf)
        in_maps.append(m)

    res = run_bass_kernel_spmd(nc, in_maps, list(range(N_CORES))).results

    ov = np.zeros((B, D, S), np.float32)
    oi = np.zeros((B, D, S), np.float32)
    for c in range(N_CORES):
        ov += res[c]["o_vis"].astype(np.float32)
        oi += res[c]["o_inf"].astype(np.float32)
    out_vis = ov.transpose(0, 2, 1) + np.asarray(b_o_vis)[None, None, :]
    out_inf = oi.transpose(0, 2, 1) + np.asarray(b_o_inf)[None, None, :]
    return (out_vis.astype(np.float32), out_inf.astype(np.float32))


# revision 67
# speedup vs baseline: 1.1168x; 1.0153x over previous
"""Trainium2 Bass kernel for nn_MultiHeadCrossAttention.

Reference computation (B=2, S=2048, D=1024, H=16, HD=64):
  Qv,Kv,Vv = vis @ W_{q,k,v}_vis + b ; Qi,Ki,Vi = inf @ W_{q,k,v}_inf + b
  out_inf = softmax(Qv Ki^T / 8) Vi @ W_o_inf + b_o_inf
  out_vis = softmax(Qi Kv^T / 8) Vv @ W_o_vis + b_o_vis

Sharding: tensor-parallel over the 16 heads; core c owns heads 2c, 2c+1
(columns 128c:128c+128 of the QKV projections, rows of W_o). Each core
computes a full-shape partial of both outputs; the host sums the 8
partials (the "all-reduce after fc_out") and adds the output biases.

Device dataflow is fully transposed (token dim on the free axis):
  QT/KT/VT[j, t] = W.T @ X^T        (W stationary, X^T moving, 8 K-tiles)
  V = transpose(VT) via PE          (+ ones column -> V_aug [128k, 65])
  S^T[k, q]      = KT.T @ QT        (per head, K=64, row-group packed:
                                     both heads' matmuls run concurrently
                                     in disjoint PE row groups)
  E = exp(0.125 * S^T)              (ScalarE, PSUM -> SBUF bf16)
  PV[hd+1, q]    = V_aug.T @ E      (K=128; row 64 = softmax denominator)
  A^T[j, q]      = PV[:64] * bcast(1/PV[64])
  OUT^T[m, t]    = Wo.T @ A^T       (K=128, 8 m-tiles, bf16 partials out)

Schedule: ONE global software pipeline.  The ScalarE exp stream (256
ACTIVATEs x ~1.05us engine-busy = the largest single-engine load) is the
master sequence; all other PE work is filler popped into PE slack
between attention matmuls.  Filler lives in three queues ordered by
consumption time:
  fq_kv: K/V projections + V transposes (a phase's kv side is consumed
         entirely within its FIRST query tile, so these are urgent),
  fq_q:  Q projections (consumed one tile per query tile — relaxed;
         loads its own copy of the input tiles so the two queues share
         no SBUF ring and can pop independently),
  fq_wo: deferred Wo output-projection tiles (no deadline).
ensure() force-pops a queue up to a dependency watermark before any
attention matmul that consumes it is emitted, which both guarantees
deadlock-freedom (every engine queue's order embeds in one global
topological order) and makes the schedule self-correcting.

PSUM budget (8 banks): scores pair [128,2,512]f32 x2 bufs = 4, PV pair
[65,512]f32 x2 = 2, projection accumulator = 1, Wo-out/transpose = 1.
"""

import sys
from collections import deque

for _p in ("/opt/trn_rl_repo", "/root/.axon_site/_ro/trn_rl_repo"):
    if _p not in sys.path:
        sys.path.append(_p)

import numpy as np
import ml_dtypes

import concourse.bass as bass
import concourse.tile as tile
from concourse import bacc, mybir
from concourse.masks import make_identity

F32 = mybir.dt.float32
BF16 = mybir.dt.bfloat16
EXP = mybir.ActivationFunctionType.Exp

B, S, D, H = 2, 2048, 1024, 16
HD = 64
JC = 128          # head dims per core (2 heads x 64)
N_CORES = 8
NT = 512          # token tile (moving dim) for projections
NQ = 512          # query tile for attention
DKT = D // 128    # 8 contraction tiles for projections
SCALE = 1.0 / np.sqrt(HD)

NTT = S // NT     # 4 token tiles per batch
NQT = S // NQ     # 4 query tiles
NKT = S // 128    # 16 key tiles
NMT = D // 128    # 8 output m-tiles

PV_LAG = 3        # PV trails the scores/exp stream by 3 key tiles
POPS_PER_TILE = 3.0


def build_kernel():
    nc = bacc.Bacc()

    visT = nc.dram_tensor("visT", [B, D, S], BF16, kind="ExternalInput")
    infT = nc.dram_tensor("infT", [B, D, S], BF16, kind="ExternalInput")
    w_in = {}
    b_in = {}
    for st in ("v", "i"):
        for p in ("q", "k", "v"):
            w_in[p + st] = nc.dram_tensor(f"w_{p}{st}", [D, JC], BF16, kind="ExternalInput")
            b_in[p + st] = nc.dram_tensor(f"b_{p}{st}", [JC], F32, kind="ExternalInput")
    w_ov = nc.dram_tensor("w_ov", [JC, D], BF16, kind="ExternalInput")
    w_oi = nc.dram_tensor("w_oi", [JC, D], BF16, kind="ExternalInput")
    # bf16 partials: the host sums them in f32, so the only cost is one
    # rounding of each partial (~3e-4 relative on the summed output).
    o_vis = nc.dram_tensor("o_vis", [B, D, S], BF16, kind="ExternalOutput")
    o_inf = nc.dram_tensor("o_inf", [B, D, S], BF16, kind="ExternalOutput")

    with tile.TileContext(nc) as tc:
        with (
            tc.tile_pool(name="const", bufs=1) as cpool,
            tc.tile_pool(name="wpool", bufs=1) as wpool,
            tc.tile_pool(name="proj", bufs=1) as projpool,
            tc.tile_pool(name="xin", bufs=2) as xpool,
            tc.tile_pool(name="esb", bufs=4) as epool,
            tc.tile_pool(name="small", bufs=2) as spool,
            tc.tile_pool(name="outst", bufs=4) as opool,
            tc.tile_pool(name="ps", bufs=1, space="PSUM") as ps,
        ):
            ident = cpool.tile([128, 128], BF16)
            make_identity(nc, ident[:])

            # Pre-load the exp table set (~2.7us) under the prologue DMAs
            # instead of on the first real scores tile.
            warm = cpool.tile([1, 1], F32, tag="warm", name="warm")
            nc.vector.memset(warm[:], 0.0)
            nc.scalar.activation(warm[:], warm[:], EXP)

            def pe_warm_spin():
                # Dummy matmuls bridging the ~13us wait for the first
                # input DMAs: keeps the HAM activity window tripped so
                # the first real matmuls run at 2.4GHz, not 1.2GHz.
                # Alternate two PSUM banks so the WAW chain still
                # pipelines.
                wa = ps.tile([128, 128], F32, tag="acc", bufs=1,
                             name="warmpa")
                wb = ps.tile([65, 128], F32, tag="pv0", name="warmpb")
                for _ in range(36):
                    nc.tensor.matmul(wa[:], ident[:], ident[:],
                                     start=True, stop=True)
                    nc.tensor.matmul(wb[:], ident[:, 0:65], ident[:],
                                     start=True, stop=True)

            # Weight/bias DMAs emitted lazily at first use.
            _w_tiles, _b_tiles, _wo_tiles = {}, {}, {}

            def w_sb_get(key, eng=None):
                # Prologue weight loads post from the still-idle qACT DGE
                # queue so they run in parallel with qSP's input postings
                # (descriptor postings are the scarce serial resource).
                if key not in _w_tiles:
                    e = eng or nc.sync
                    t = wpool.tile([128, DKT, JC], BF16, tag=f"w_{key}",
                                   name=f"w_{key}")
                    src = w_in[key].rearrange("(kt p) j -> p kt j", p=128)
                    e.dma_start(t[:], src)
                    _w_tiles[key] = t
                return _w_tiles[key]

            # All six QKV biases in one batch: six contiguous [1,128] row
            # DMAs (a [128,1] gather costs ~3.5us of descriptor posting
            # each), one cast, ONE K=6 matmul against I6 to flip them to
            # [128, 6], one copy out.  bias_sb_get returns column slices.
            _BKEYS = ("ki", "vi", "qv", "kv", "vv", "qi")
            _bias_state = {}

            def load_biases():
                rows = cpool.tile([6, JC], F32, tag="brows", name="brows")
                for idx, key in enumerate(_BKEYS):
                    nc.scalar.dma_start(rows[idx:idx + 1, :],
                                        b_in[key][:].unsqueeze(0))
                rows16 = cpool.tile([6, JC], BF16, tag="brows16",
                                    name="brows16")
                nc.vector.tensor_copy(rows16[:], rows[:])
                tp = ps.tile([JC, 6], F32, tag="pwo", bufs=1, name="btp")
                nc.tensor.matmul(tp[:], rows16[:], ident[0:6, 0:6],
                                 start=True, stop=True)
                ball = cpool.tile([JC, 6], F32, tag="ball", name="ball")
                nc.vector.tensor_copy(ball[:], tp[:])
                _bias_state["ball"] = ball

            def bias_sb_get(key):
                idx = _BKEYS.index(key)
                return _bias_state["ball"][:, idx:idx + 1]

            def wo_sb_get(key):
                if key not in _wo_tiles:
                    wd = {"v": w_ov, "i": w_oi}[key]
                    t = wpool.tile([JC, NMT, 128], BF16, tag=f"wo_{key}",
                                   name=f"wo_{key}")
                    nc.sync.dma_start(
                        t[:], wd.rearrange("j (mt m) -> j mt m", m=128))
                    _wo_tiles[key] = t
                return _wo_tiles[key]

            xT = {"v": visT, "i": infT}
            o_dram = {"v": o_vis, "i": o_inf}

            # ---- per-(b, st) projection output tiles --------------------
            # Each tag ring has bufs=2 and exactly two allocations (b=0,
            # b=1), so the two batches never alias.
            qt_sb, kt_sb, vt_sb, vaug_sb = {}, {}, {}, {}

            def get_proj_tiles(b, st):
                key = (b, st)
                if key not in qt_sb:
                    qt_sb[key] = projpool.tile([JC, S], BF16, tag=f"QT_{st}",
                                               bufs=2, name=f"QT_{st}{b}")
                    kt_sb[key] = projpool.tile([JC, S], BF16, tag=f"KT_{st}",
                                               bufs=2, name=f"KT_{st}{b}")
                    vt_sb[key] = projpool.tile([JC, S], BF16, tag=f"VT_{st}",
                                               bufs=2, name=f"VT_{st}{b}")
                    vaug_sb[key] = projpool.tile([128, NKT, 130], BF16,
                                                 tag=f"Vaug_{st}", bufs=2,
                                                 name=f"Vaug_{st}{b}")

            # ---- filler task queues -------------------------------------
            # Credit-based popping: each attention tile adds
            # POPS_PER_TILE of PE-work credit; popped units (including
            # ensure()-forced ones) consume it, so dependency bursts
            # automatically suppress later discretionary pops.  Units are
            # ATOMIC: a whole PSUM accumulation group lives in one unit,
            # so ring-shared PSUM tags are never interleaved mid-group.
            fq_kv = deque()
            fq_q = deque()
            fq_wo = deque()
            _done = set()
            _credit = [0.0]
            _rr = [0]

            def _pop_one(q):
                w, fn, provides = q.popleft()
                fn()
                if provides is not None:
                    _done.add(provides)
                _credit[0] -= max(w, 0.25)

            def add_credit(n):
                _credit[0] = min(_credit[0] + n, 8.0)

            def pop_filler():
                while _credit[0] > 0 and (fq_kv or fq_q or fq_wo):
                    # Drain the Wo backlog preferentially once it builds,
                    # else it all lands in a serial post-attention tail.
                    if len(fq_wo) > 2:
                        _pop_one(fq_wo)
                        continue
                    for _ in range(3):
                        q = (fq_kv, fq_q, fq_wo)[_rr[0] % 3]
                        _rr[0] += 1
                        if q:
                            _pop_one(q)
                            break

            def ensure(key):
                q = fq_q if (key[0] == "P" and key[3] == "q") else fq_kv
                while key not in _done:
                    assert q, f"dependency {key} not queued"
                    _pop_one(q)

            def flush_filler():
                for q in (fq_kv, fq_q, fq_wo):
                    while q:
                        _pop_one(q)

            # ---- projection filler units --------------------------------
            # side: "kv" tiles feed K/V projections, "q" tiles feed Q.
            # Separate tags so the two queues never share an SBUF ring
            # (the q side re-loads the inputs; DMA bandwidth is cheap).
            xt_live = {}
            acc_live = {}

            def u_dma_x(side, b, st, tt, eng=None):
                # Per-kt 2D transfers: each [128, 512] piece is one clean
                # descriptor posting with its own completion, so the
                # projection matmuls for early kt can start while later
                # pieces are still in flight.
                def fn():
                    e = eng or nc.sync
                    xt = xpool.tile([128, DKT, NT], BF16, tag=f"x{side}_{st}",
                                    bufs=(3 if side == "kv" else 2),
                                    name=f"x{side}_{st}")
                    src = xT[st].rearrange("bb (kt p) t -> bb p kt t", p=128)[
                        b, :, :, tt * NT:(tt + 1) * NT]
                    e.dma_start(xt[:], src)
                    xt_live[(side, b, st, tt)] = xt
                return (0.0, fn, None)

            def u_proj_kv_half(b, st, tt, p, half):
                # half a K/V projection group (4 accumulating matmuls).
                # The "acc" bank is used ONLY by these kv units (plus
                # tail-time Wo units once the proj queues are empty), and
                # both halves sit adjacent in fq_kv, so no other acc
                # allocation can interleave the accumulation group.
                def fn():
                    bias = bias_sb_get(p + st)
                    xt = xt_live[("kv", b, st, tt)]
                    w = w_sb_get(p + st)
                    if half == 0:
                        acc_live["kv"] = ps.tile([128, NT], F32, tag="acc",
                                                 bufs=1, name="acc")
                    acc = acc_live["kv"]
                    for kt in range(half * 4, half * 4 + 4):
                        nc.tensor.matmul(
                            acc[:], w[:, kt, :], xt[:, kt, :],
                            start=(kt == 0), stop=(kt == DKT - 1),
                        )
                    if half == 1:
                        dst = {"k": kt_sb[(b, st)], "v": vt_sb[(b, st)]}[p]
                        nc.vector.tensor_scalar_add(
                            dst[:, tt * NT:(tt + 1) * NT], acc[:], bias)
                return (4.0, fn,
                        ("P", b, st, p, tt) if half == 1 else None)

            def u_proj_q(b, st, tt):
                # one full Q projection group, atomic, on the "pwo" ring
                # (shared with Wo/transpose units, which are also atomic).
                def fn():
                    bias = bias_sb_get("q" + st)
                    xt = xt_live[("q", b, st, tt)]
                    w = w_sb_get("q" + st)
                    acc = ps.tile([128, NT], F32, tag="pwo", bufs=1,
                                  name="accq")
                    for kt in range(DKT):
                        nc.tensor.matmul(
                            acc[:], w[:, kt, :], xt[:, kt, :],
                            start=(kt == 0), stop=(kt == DKT - 1),
                        )
                    nc.vector.tensor_scalar_add(
                        qt_sb[(b, st)][:, tt * NT:(tt + 1) * NT], acc[:],
                        bias)
                return (8.0, fn, ("P", b, st, "q", tt))

            def u_vaug_init(b, st):
                def fn():
                    Vaug = vaug_sb[(b, st)]
                    nc.vector.memset(Vaug[:, :, 64:65], 1.0)
                    nc.vector.memset(Vaug[:, :, 129:130], 1.0)
                return (0.0, fn, None)

            def u_tr(b, st, k16):
                # PE transpose of one 128-key block of VT into V_aug
                def fn():
                    VT = vt_sb[(b, st)]
                    Vaug = vaug_sb[(b, st)]
                    trp = ps.tile([128, 128], BF16, tag="pwo", bufs=1,
                                  name="trp")
                    nc.tensor.transpose(
                        trp[:], VT[:, k16 * 128:(k16 + 1) * 128], ident[:])
                    nc.vector.tensor_copy(Vaug[:, k16, 0:64], trp[:, 0:64])
                    nc.vector.tensor_copy(Vaug[:, k16, 65:129], trp[:, 64:128])
                return (0.7, fn, ("T", b, st, k16))

            def queue_kv_block(b, st, skip_dma0=False):
                """K/V projections + transposes for (b, st), k16-ordered."""
                get_proj_tiles(b, st)
                fq_kv.append(u_vaug_init(b, st))
                # prefetch depth 3: post three token tiles' loads up
                # front (each ~13us in flight; consumed every ~4.6us)
                if not skip_dma0:
                    fq_kv.append(u_dma_x("kv", b, st, 0))
                fq_kv.append(u_dma_x("kv", b, st, 1))
                fq_kv.append(u_dma_x("kv", b, st, 2))
                for tt in range(NTT):
                    if tt == 1:
                        fq_kv.append(u_dma_x("kv", b, st, 3))
                    for p in ("k", "v"):
                        fq_kv.append(u_proj_kv_half(b, st, tt, p, 0))
                        fq_kv.append(u_proj_kv_half(b, st, tt, p, 1))
                    for k16 in range(tt * 4, tt * 4 + 4):
                        fq_kv.append(u_tr(b, st, k16))

            def queue_q_block(b, st, skip_dma0=False):
                # single-buffered xq ring: each tt's DMA must follow the
                # previous tt's projection (WAR), and Q is consumed only
                # once per query tile (~18us apart) so depth 1 suffices.
                get_proj_tiles(b, st)
                if not skip_dma0:
                    fq_q.append(u_dma_x("q", b, st, 0))
                for tt in range(NTT):
                    fq_q.append(u_proj_q(b, st, tt))
                    if tt + 1 < NTT:
                        fq_q.append(u_dma_x("q", b, st, tt + 1))

            # ---- Wo output-projection filler units ----------------------
            _wo_alt = [0]

            def u_wo(wo, mt, AT_, qsl_, od_, b_):
                def fn():
                    # Once the projection queues are drained the "acc"
                    # bank is free for good; alternating the two banks
                    # lets tail Wo matmuls double-buffer instead of
                    # stalling on each DVE drain.
                    if not (fq_kv or fq_q):
                        _wo_alt[0] ^= 1
                        tag = ("pwo", "acc")[_wo_alt[0]]
                    else:
                        tag = "pwo"
                    po = ps.tile([128, NQ], F32, tag=tag, bufs=1, name="po")
                    nc.tensor.matmul(po[:], wo[:, mt, :], AT_[:, qsl_],
                                     start=True, stop=True)
                    ot = opool.tile([128, NQ], BF16, tag="ot", name="ot")
                    nc.vector.tensor_copy(ot[:], po[:])
                    nc.sync.dma_start(
                        od_[b_, mt * 128:(mt + 1) * 128, qsl_], ot[:])
                return (1.0, fn, None)

            # ---- attention phase ----------------------------------------
            def attention(b, qst, kvst, ost):
                QT = qt_sb[(b, qst)]
                KTt = kt_sb[(b, kvst)]
                Vaug = vaug_sb[(b, kvst)]
                AT = projpool.tile([JC, S], BF16, tag=f"AT_{ost}", bufs=2)
                wo = wo_sb_get(ost)
                for qt in range(NQT):
                    qsl = slice(qt * NQ, (qt + 1) * NQ)
                    ensure(("P", b, qst, "q", qt))
                    pv0 = ps.tile([65, NQ], F32, tag="pv0")
                    pv1 = ps.tile([65, NQ], F32, tag="pv1")
                    es = [None] * NKT

                    def stage_s(k16, _es=es, _K=KTt, _Q=QT, _q=qsl):
                        ensure(("P", b, kvst, "k", k16 // 4))
                        ksl = slice(k16 * 128, (k16 + 1) * 128)
                        sp = ps.tile([128, 2, NQ], F32, tag="spair", bufs=2)
                        nc.tensor.matmul(sp[:, 0, :], _K[0:64, ksl],
                                         _Q[0:64, _q], start=True, stop=True)
                        nc.tensor.matmul(sp[:, 1, :], _K[64:128, ksl],
                                         _Q[64:128, _q], start=True, stop=True)
                        e01 = epool.tile([128, 2, NQ], BF16, tag="e01")
                        nc.scalar.activation(e01[:], sp[:], EXP, scale=SCALE)
                        _es[k16] = e01

                    def stage_pv(k16, _es=es, _V=Vaug, _pv0=pv0, _pv1=pv1):
                        ensure(("T", b, kvst, k16))
                        e01 = _es[k16]
                        nc.tensor.matmul(_pv0[:], _V[:, k16, 0:65],
                                         e01[:, 0, :],
                                         start=(k16 == 0), stop=(k16 == NKT - 1))
                        nc.tensor.matmul(_pv1[:], _V[:, k16, 65:130],
                                         e01[:, 1, :],
                                         start=(k16 == 0), stop=(k16 == NKT - 1))
                        _es[k16] = None

                    for k16 in range(NKT):
                        add_credit(POPS_PER_TILE)
                        stage_s(k16)
                        if k16 >= PV_LAG:
                            stage_pv(k16 - PV_LAG)
                        if k16 == 8 and qt + 1 < NQT:
                            # pull the next query tile's Q projection in
                            # mid-stream so the qt boundary doesn't stall
                            # on its atomic 8-matmul group
                            ensure(("P", b, qst, "q", qt + 1))
                        pop_filler()
                    for k16 in range(NKT - PV_LAG, NKT):
                        add_credit(1.0)
                        stage_pv(k16)
                        pop_filler()
                    # normalize: A^T = PV[:64] * bcast(1 / PV[64]).
                    # Denominators bounce PSUM->SBUF (raf can't read PSUM
                    # on HW), one fast-reciprocal pass, one combined
                    # partition broadcast for both heads.
                    # Interleave the two heads' chains so head1's gpsimd
                    # broadcast overlaps head0's DVE multiply.
                    den = spool.tile([1, 2, NQ], F32, tag="den", bufs=1)
                    rec = spool.tile([1, 2, NQ], F32, tag="rec", bufs=1)
                    rb = spool.tile([64, 2, NQ], F32, tag="rb", bufs=1)
                    nc.vector.tensor_copy(den[0:1, 0, :], pv0[64:65, :])
                    nc.vector.reciprocal_approx_fast(rec[0:1, 0, :],
                                                     den[0:1, 0, :])
                    nc.gpsimd.partition_broadcast(rb[:, 0, :], rec[0:1, 0, :])
                    nc.vector.tensor_copy(den[0:1, 1, :], pv1[64:65, :])
                    nc.vector.reciprocal_approx_fast(rec[0:1, 1, :],
                                                     den[0:1, 1, :])
                    nc.vector.tensor_mul(AT[0:64, qsl], pv0[0:64, :],
                                         rb[:, 0, :])
                    nc.gpsimd.partition_broadcast(rb[:, 1, :], rec[0:1, 1, :])
                    nc.vector.tensor_mul(AT[64:128, qsl], pv1[0:64, :],
                                         rb[:, 1, :])
                    for mt in range(NMT):
                        fq_wo.append(u_wo(wo, mt, AT, qsl, o_dram[ost], b))

            # ---- master sequence ----------------------------------------
            # Kick the gating DMAs (first input tiles, first weights)
            # immediately so the first scores matmul isn't waiting on a
            # cold queue.
            # Startup postings fan out across three DGE queues: kv input
            # on qSP, first q-side input on the (still idle) qACT, and
            # weights/biases on qDVE.
            get_proj_tiles(0, "i")
            get_proj_tiles(0, "v")
            u_dma_x("kv", 0, "i", 0)[1]()          # qSP
            load_biases()                          # qACT (tiny, first)
            u_dma_x("q", 0, "v", 0, eng=nc.scalar)[1]()  # qACT
            for wk in ("ki", "vi", "qv"):
                w_sb_get(wk)                       # qSP, behind xkv
            pe_warm_spin()

            queue_kv_block(0, "i", skip_dma0=True)
            queue_kv_block(0, "v")
            queue_q_block(0, "v", skip_dma0=True)
            queue_q_block(0, "i")

            attention(0, "v", "i", "i")
            queue_kv_block(1, "i")
            queue_q_block(1, "v")
            attention(0, "i", "v", "v")
            queue_kv_block(1, "v")
            queue_q_block(1, "i")
            attention(1, "v", "i", "i")
            attention(1, "i", "v", "v")

            flush_filler()

    nc.compile()
    return nc


_NC_CACHE = None


def _get_nc():
    global _NC_CACHE
    if _NC_CACHE is None:
        _NC_CACHE = build_kernel()
    return _NC_CACHE


def kernel(vis, inf, W_q_vis, b_q_vis, W_k_vis, b_k_vis, W_v_vis, b_v_vis,
           W_q_inf, b_q_inf, W_k_inf, b_k_inf, W_v_inf, b_v_inf,
           W_o_vis, b_o_vis, W_o_inf, b_o_inf):
    from concourse.bass_utils import run_bass_kernel_spmd

    nc = _get_nc()
    bf = ml_dtypes.bfloat16
    visT = np.ascontiguousarray(np.asarray(vis).transpose(0, 2, 1)).astype(bf)
    infT = np.ascontiguousarray(np.asarray(inf).transpose(0, 2, 1)).astype(bf)

    wq = {"v": np.asarray(W_q_vis), "i": np.asarray(W_q_inf)}
    wk = {"v": np.asarray(W_k_vis), "i": np.asarray(W_k_inf)}
    wv = {"v": np.asarray(W_v_vis), "i": np.asarray(W_v_inf)}
    bq = {"v": np.asarray(b_q_vis), "i": np.asarray(b_q_inf)}
    bk = {"v": np.asarray(b_k_vis), "i": np.asarray(b_k_inf)}
    bv = {"v": np.asarray(b_v_vis), "i": np.asarray(b_v_inf)}
    wo = {"v": np.asarray(W_o_vis), "i": np.asarray(W_o_inf)}

    in_maps = []
    for c in range(N_CORES):
        sl = slice(c * JC, (c + 1) * JC)
        m = {"visT": visT, "infT": infT}
        for st in ("v", "i"):
            m[f"w_q{st}"] = np.ascontiguousarray(wq[st][:, sl]).astype(bf)
            m[f"w_k{st}"] = np.ascontiguousarray(wk[st][:, sl]).astype(bf)
            m[f"w_v{st}"] = np.ascontiguousarray(wv[st][:, sl]).astype(bf)
            m[f"b_q{st}"] = np.ascontiguousarray(bq[st][sl]).astype(np.float32)
            m[f"b_k{st}"] = np.ascontiguousarray(bk[st][sl]).astype(np.float32)
            m[f"b_v{st}"] = np.ascontiguousarray(bv[st][sl]).astype(np.float32)
        m["w_ov"] = np.ascontiguousarray(wo["v"][sl, :]).astype(bf)
        m["w_oi"] = np.ascontiguousarray(wo["i"][sl, :]).astype(bf)
        in_maps.append(m)

    res = run_bass_kernel_spmd(nc, in_maps, list(range(N_CORES))).results

    ov = np.zeros((B, D, S), np.float32)
    oi = np.zeros((B, D, S), np.float32)
    for c in range(N_CORES):
        ov += res[c]["o_vis"].astype(np.float32)
        oi += res[c]["o_inf"].astype(np.float32)
    out_vis = ov.transpose(0, 2, 1) + np.asarray(b_o_vis)[None, None, :]
    out_inf = oi.transpose(0, 2, 1) + np.asarray(b_o_inf)[None, None, :]
    return (out_vis.astype(np.float32), out_inf.astype(np.float32))
